# revision 6
# baseline (speedup 1.0000x reference)
"""GroupPointNet kernel for 8 Trainium2 NeuronCores.

Strategy:
- FPS + KNN index selection run on host in jax-CPU with the exact reference
  op order (argmax/top-k tie-breaking must match the oracle bit-for-bit;
  a diverged FPS trajectory corrupts every downstream output position).
- The dense pipeline — 3x (1x1 conv matmul + LeakyReLU + BatchNorm) + max-pool
  over K — runs on the 8 cores, data-parallel over the B*M (b,m) groups,
  with in-kernel AllReduce for the global BatchNorm statistics.
"""

import numpy as np

SAMPLE_RATIO = 0.25
K = 20
SLOPE = 0.2
EPS = 1e-5

B, N, C = 4, 8192, 64
M = int(N * SAMPLE_RATIO)          # 2048
L = B * M * K                      # 163840 columns, ordered (b, m, k)
N_CORES = 8
GROUPS = B * M                     # 8192 (b,m) groups
GPC = GROUPS // N_CORES            # 1024 groups per core
LC = GPC * K                       # 20480 columns per core
# column chunks per core: multiples of K so max-pool groups never straddle
CHUNK = 500                        # 25 groups
CHUNKS = [(i * CHUNK, CHUNK) for i in range(LC // CHUNK)]
_rem = LC - (LC // CHUNK) * CHUNK
if _rem:
    CHUNKS.append(((LC // CHUNK) * CHUNK, _rem))
NCH = len(CHUNKS)

_CACHE = {}


def _get_host_fns():
    """Jitted FPS + KNN (reference-identical numerics), built once."""
    if "hostfns" in _CACHE:
        return _CACHE["hostfns"]
    import jax
    import jax.numpy as jnp
    from jax import lax

    cpu = jax.devices("cpu")[0]

    def fps(p, m):
        B_, N_, _ = p.shape

        def step(carry, _):
            dist, last_idx = carry
            last_pt = jnp.take_along_axis(p, last_idx[:, None, None], axis=1)
            d = jnp.sum((p - last_pt) ** 2, axis=-1)
            dist = jnp.minimum(dist, d)
            nxt = jnp.argmax(dist, axis=1).astype(jnp.int32)
            return (dist, nxt), last_idx

        dist0 = jnp.full((B_, N_), 1e10, dtype=p.dtype)
        idx0 = jnp.zeros((B_,), dtype=jnp.int32)
        _, idxs = lax.scan(step, (dist0, idx0), None, length=m)
        return jnp.transpose(idxs)

    def knn_idx(q, p, k):
        d = (jnp.sum(q * q, -1)[:, :, None]
             + jnp.sum(p * p, -1)[:, None, :]
             - 2.0 * jnp.einsum('bmd,bnd->bmn', q, p))
        _, idx = lax.top_k(-d, k)
        return idx

    jfps = jax.jit(fps, static_argnums=1)
    jknn = jax.jit(knn_idx, static_argnums=2)
    _CACHE["hostfns"] = (jax, jnp, cpu, jfps, jknn)
    return _CACHE["hostfns"]


def _host_indices(p_np):
    """FPS + KNN with reference-identical numerics on jax CPU."""
    jax, jnp, cpu, jfps, jknn = _get_host_fns()
    with jax.default_device(cpu):
        p = jnp.asarray(p_np)
        idx = jfps(p, M)
        p1 = jnp.take_along_axis(p, idx[:, :, None], axis=1)
        nidx = jknn(p1, p, K)
        return np.asarray(p1), np.asarray(nidx)


def _apply_drain_patch():
    """This walrus build rejects >1 sync wait on a CTRL-format instruction;
    split the TileContext kernel-tail drain's waits across single-wait NoOps."""
    import concourse.tile as tile_mod
    import concourse.mybir as mybir
    from concourse.vector_clock import ScopedClock

    def _split_drain_and_barrier(self, tick_clock, wait_clock):
        nc = self.nc
        drain_inst = nc.sync.drain()
        wait_clock.add_sem_waits(
            drain_inst.ins, ScopedClock({None: tick_clock.global_clock})
        )
        si = drain_inst.ins.sync_info
        if si is not None and si.on_wait and len(si.on_wait) > 1:
            waits = list(si.on_wait)
            si.on_wait = waits[:1]
            for w in waits[1:]:
                nop = nc.sync.nop(nofuse=True)
                nop.ins.sync_info = mybir.SyncInfo(on_wait=[w], on_update=[])
        nc.all_engine_barrier()
        assert self.sems is not None
        popped = nc._tile_sem_poison_stack.pop()
        assert popped is self._sem_poison
        nc.clear_and_free_semaphores(list(self.sems.allocated().values()))
        nc.all_engine_barrier()

    tile_mod.TileContext._drain_and_barrier = _split_drain_and_barrier


def _split_multi_waits(nc):
    """This walrus build allows only ONE sync wait per instruction (any
    format). Hoist extra waits onto same-engine NoOps inserted just before
    the owning instruction — in-order engines make this equivalent."""
    import concourse.mybir as mybir

    cnt = 0
    for f in nc.m.functions:
        for blk in f.blocks:
            changed = False
            out = []
            for ins in blk.instructions:
                si = ins.sync_info
                if si is not None and si.on_wait and len(si.on_wait) > 1:
                    waits = list(si.on_wait)
                    for w in waits[:-1]:
                        nop = mybir.InstNoOp(name=f"wsplit_{cnt}", ins=[], outs=[])
                        cnt += 1
                        nop.engine = ins.engine
                        nop.sync_info = mybir.SyncInfo(on_wait=[w], on_update=[])
                        out.append(nop)
                    si.on_wait = waits[-1:]
                    changed = True
                out.append(ins)
            if changed:
                blk.instructions = out
    return cnt


def _build_nc():
    import concourse.bass as bass
    import concourse.mybir as mybir
    import concourse.tile as tile

    _apply_drain_patch()
    dt = mybir.dt.float32
    Alu = mybir.AluOpType
    Act = mybir.ActivationFunctionType

    nc = bass.Bass("TRN2", target_bir_lowering=False, debug=False,
                   num_devices=N_CORES)

    xc = nc.dram_tensor("xc", [6, LC], dt, kind="ExternalInput")
    w1t = nc.dram_tensor("w1t", [6, C], dt, kind="ExternalInput")
    w2t = nc.dram_tensor("w2t", [C, C], dt, kind="ExternalInput")
    w3t = nc.dram_tensor("w3t", [C, C], dt, kind="ExternalInput")
    gb = nc.dram_tensor("gb", [C, 6], dt, kind="ExternalInput")
    y = nc.dram_tensor("y", [C, GPC], dt, kind="ExternalOutput")

    inv_count = 1.0 / float(L)

    with tile.TileContext(nc) as tc:
        with (
            tc.tile_pool(name="const", bufs=1) as cpool,
            tc.tile_pool(name="slab", bufs=1) as slab,
            tc.tile_pool(name="chunk", bufs=3) as ch,
            tc.tile_pool(name="psum", bufs=4, space="PSUM") as pp,
            tc.tile_pool(name="stats", bufs=1) as sp,
            tc.tile_pool(name="dram", bufs=1, space="DRAM") as dram,
        ):
            w1s = cpool.tile([6, C], dt, tag="w1")
            w2s = cpool.tile([C, C], dt, tag="w2")
            w3s = cpool.tile([C, C], dt, tag="w3")
            gbs = cpool.tile([C, 6], dt, tag="gb")
            nc.sync.dma_start(w1s[:], w1t[:])
            nc.sync.dma_start(w2s[:], w2t[:])
            nc.sync.dma_start(w3s[:], w3t[:])
            nc.sync.dma_start(gbs[:], gb[:])

            z1 = slab.tile([C, LC], dt, tag="slabA")
            z2 = slab.tile([C, LC], dt, tag="slabB")

            ssum = sp.tile([C, NCH], dt, tag="ssum1")
            qsum = sp.tile([C, NCH], dt, tag="qsum1")

            def stats_and_scale(layer, s_tile, q_tile, g_col, b_col):
                """Reduce per-chunk stats, AllReduce across cores, produce
                per-channel (scale, bias) implementing BN."""
                st = sp.tile([C, 2], dt, tag=f"st{layer}")
                nc.vector.tensor_reduce(st[:, 0:1], s_tile[:, :NCH],
                                        mybir.AxisListType.X, Alu.add)
                nc.vector.tensor_reduce(st[:, 1:2], q_tile[:, :NCH],
                                        mybir.AxisListType.X, Alu.add)
                cc_in = dram.tile([C, 2], dt, tag=f"ccin{layer}")
                cc_out = dram.tile([C, 2], dt, tag=f"ccout{layer}")
                nc.sync.dma_start(cc_in[:], st[:])
                nc.gpsimd.collective_compute(
                    "AllReduce", Alu.add,
                    replica_groups=[list(range(N_CORES))],
                    ins=[cc_in[:]], outs=[cc_out[:]],
                )
                gst = sp.tile([C, 2], dt, tag=f"gst{layer}")
                nc.sync.dma_start(gst[:], cc_out[:])
                mean = sp.tile([C, 1], dt, tag=f"mean{layer}")
                ex2 = sp.tile([C, 1], dt, tag=f"ex2{layer}")
                var = sp.tile([C, 1], dt, tag=f"var{layer}")
                sd = sp.tile([C, 1], dt, tag=f"sd{layer}")
                inv = sp.tile([C, 1], dt, tag=f"inv{layer}")
                scale = sp.tile([C, 1], dt, tag=f"scale{layer}")
                bias = sp.tile([C, 1], dt, tag=f"bias{layer}")
                nc.vector.tensor_scalar_mul(mean[:], gst[:, 0:1], inv_count)
                nc.vector.tensor_scalar_mul(ex2[:], gst[:, 1:2], inv_count)
                nc.vector.tensor_mul(var[:], mean[:], mean[:])
                nc.vector.tensor_sub(var[:], ex2[:], var[:])
                nc.vector.tensor_scalar_add(var[:], var[:], EPS)
                nc.scalar.activation(sd[:], var[:], Act.Sqrt, bias=0.0)
                nc.vector.reciprocal(inv[:], sd[:])
                nc.vector.tensor_mul(scale[:], g_col, inv[:])
                nc.vector.tensor_mul(bias[:], mean[:], scale[:])
                nc.vector.tensor_sub(bias[:], b_col, bias[:])
                return scale, bias

            # ---- layer 1: conv1 + leaky + stats (input streamed from DRAM)
            for i, (off, w) in enumerate(CHUNKS):
                xt = ch.tile([6, CHUNK], dt, tag="xin")
                nc.sync.dma_start(xt[:, :w], xc[:, off:off + w])
                ps = pp.tile([C, CHUNK], dt, tag="ps")
                nc.tensor.matmul(ps[:, :w], w1s[:], xt[:, :w],
                                 start=True, stop=True)
                zr = ch.tile([C, CHUNK], dt, tag="zraw")
                nc.scalar.activation(zr[:, :w], ps[:, :w], Act.Copy, bias=0.0)
                nc.vector.scalar_tensor_tensor(
                    z1[:, off:off + w], zr[:, :w], SLOPE, zr[:, :w],
                    Alu.mult, Alu.max, accum_out=ssum[:, i:i + 1])
                scr = ch.tile([C, CHUNK], dt, tag="scr")
                nc.scalar.activation(scr[:, :w], z1[:, off:off + w], Act.Square,
                                     accum_out=qsum[:, i:i + 1])

            sc1, bi1 = stats_and_scale(1, ssum, qsum, gbs[:, 0:1], gbs[:, 1:2])

            ssum2 = sp.tile([C, NCH], dt, tag="ssum2")
            qsum2 = sp.tile([C, NCH], dt, tag="qsum2")

            # ---- layer 2: BN1-apply + conv2 + leaky + stats
            for i, (off, w) in enumerate(CHUNKS):
                xt = ch.tile([C, CHUNK], dt, tag="xbn")
                nc.vector.tensor_scalar(xt[:, :w], z1[:, off:off + w],
                                        sc1[:], bi1[:], Alu.mult, Alu.add)
                ps = pp.tile([C, CHUNK], dt, tag="ps")
                nc.tensor.matmul(ps[:, :w], w2s[:], xt[:, :w],
                                 start=True, stop=True)
                zr = ch.tile([C, CHUNK], dt, tag="zraw")
                nc.scalar.activation(zr[:, :w], ps[:, :w], Act.Copy, bias=0.0)
                nc.vector.scalar_tensor_tensor(
                    z2[:, off:off + w], zr[:, :w], SLOPE, zr[:, :w],
                    Alu.mult, Alu.max, accum_out=ssum2[:, i:i + 1])
                scr = ch.tile([C, CHUNK], dt, tag="scr")
                nc.scalar.activation(scr[:, :w], z2[:, off:off + w], Act.Square,
                                     accum_out=qsum2[:, i:i + 1])

            sc2, bi2 = stats_and_scale(2, ssum2, qsum2, gbs[:, 2:3], gbs[:, 3:4])

            ssum3 = sp.tile([C, NCH], dt, tag="ssum3")
            qsum3 = sp.tile([C, NCH], dt, tag="qsum3")
            z3 = slab.tile([C, LC], dt, tag="slabA")  # reuse z1's slot

            # ---- layer 3: BN2-apply + conv3 + leaky + stats
            for i, (off, w) in enumerate(CHUNKS):
                xt = ch.tile([C, CHUNK], dt, tag="xbn")
                nc.vector.tensor_scalar(xt[:, :w], z2[:, off:off + w],
                                        sc2[:], bi2[:], Alu.mult, Alu.add)
                ps = pp.tile([C, CHUNK], dt, tag="ps")
                nc.tensor.matmul(ps[:, :w], w3s[:], xt[:, :w],
                                 start=True, stop=True)
                zr = ch.tile([C, CHUNK], dt, tag="zraw")
                nc.scalar.activation(zr[:, :w], ps[:, :w], Act.Copy, bias=0.0)
                nc.vector.scalar_tensor_tensor(
                    z3[:, off:off + w], zr[:, :w], SLOPE, zr[:, :w],
                    Alu.mult, Alu.max, accum_out=ssum3[:, i:i + 1])
                scr = ch.tile([C, CHUNK], dt, tag="scr")
                nc.scalar.activation(scr[:, :w], z3[:, off:off + w], Act.Square,
                                     accum_out=qsum3[:, i:i + 1])

            sc3, bi3 = stats_and_scale(3, ssum3, qsum3, gbs[:, 4:5], gbs[:, 5:6])

            # ---- BN3-apply + max-pool over K
            yslab = sp.tile([C, GPC], dt, tag="yslab")
            for i, (off, w) in enumerate(CHUNKS):
                yt = ch.tile([C, CHUNK], dt, tag="ybn")
                nc.vector.tensor_scalar(yt[:, :w], z3[:, off:off + w],
                                        sc3[:], bi3[:], Alu.mult, Alu.add)
                g0, ng = off // K, w // K
                nc.vector.tensor_reduce(
                    yslab[:, g0:g0 + ng],
                    yt[:, :w].rearrange("p (g k) -> p g k", k=K),
                    mybir.AxisListType.X, Alu.max)
            nc.sync.dma_start(y[:], yslab[:])

    _split_multi_waits(nc)
    return nc


def _build_runner(nc, n_cores):
    """Build the jitted PJRT executable ONCE (run_bass_via_pjrt rebuilds the
    jax.jit closure per call, forcing a retrace + relower every time)."""
    import jax
    import concourse.mybir as mybir
    from jax.sharding import Mesh, PartitionSpec
    from jax.experimental.shard_map import shard_map
    from concourse.bass2jax import (
        _bass_exec_p, install_neuronx_cc_hook, partition_id_tensor)

    install_neuronx_cc_hook()

    partition_name = (nc.partition_id_tensor.name
                      if nc.partition_id_tensor else None)
    in_names, out_names, out_avals, zero_outs = [], [], [], []
    for alloc in nc.m.functions[0].allocations:
        if not isinstance(alloc, mybir.MemoryLocationSet):
            continue
        name = alloc.memorylocations[0].name
        if alloc.kind == "ExternalInput":
            if name != partition_name:
                in_names.append(name)
        elif alloc.kind == "ExternalOutput":
            shape = tuple(alloc.tensor_shape)
            dtype = mybir.dt.np(alloc.dtype)
            out_avals.append(jax.core.ShapedArray(shape, dtype))
            out_names.append(name)
            zero_outs.append(np.zeros(shape, dtype))
    n_params = len(in_names)
    n_outs = len(out_avals)
    all_in_names = list(in_names) + list(out_names)
    if partition_name is not None:
        all_in_names.append(partition_name)
    donate = tuple(range(n_params, n_params + n_outs))

    def _body(*args):
        operands = list(args)
        if partition_name is not None:
            operands.append(partition_id_tensor())
        outs = _bass_exec_p.bind(
            *operands,
            out_avals=tuple(out_avals),
            in_names=tuple(all_in_names),
            out_names=tuple(out_names),
            lowering_input_output_aliases=(),
            sim_require_finite=True,
            sim_require_nnan=True,
            nc=nc,
        )
        return tuple(outs)

    devices = jax.devices()[:n_cores]
    mesh = Mesh(np.asarray(devices), ("core",))
    in_specs = (PartitionSpec("core"),) * (n_params + n_outs)
    out_specs = (PartitionSpec("core"),) * n_outs
    sharded = jax.jit(
        shard_map(_body, mesh=mesh, in_specs=in_specs, out_specs=out_specs,
                  check_rep=False),
        donate_argnums=donate, keep_unused=True)

    def run(in_maps):
        concat_in = [
            np.concatenate([np.asarray(in_maps[c][name])
                            for c in range(n_cores)], axis=0)
            for name in in_names
        ]
        concat_zeros = [
            np.zeros((n_cores * z.shape[0], *z.shape[1:]), z.dtype)
            for z in zero_outs
        ]
        out_arrs = sharded(*concat_in, *concat_zeros)
        return [
            {name: np.asarray(out_arrs[i]).reshape(
                n_cores, *out_avals[i].shape)[c]
             for i, name in enumerate(out_names)}
            for c in range(n_cores)
        ]

    return run


def kernel(p, W1, g1, b1, W2, g2, b2, W3, g3, b3):
    p = np.asarray(p, np.float32)
    p1, nidx = _host_indices(p)

    batch = np.arange(B)[:, None, None]
    grouped = p[batch, nidx]                       # [B,M,K,3]
    dp = grouped - p1[:, :, None, :]               # [B,M,K,3]
    gf = np.concatenate([dp, grouped], axis=-1)    # [B,M,K,6]
    x = np.ascontiguousarray(
        gf.reshape(L, 6).T.astype(np.float32))     # [6, L], cols (b,m,k)

    if "run" not in _CACHE:
        _CACHE["run"] = _build_runner(_build_nc(), N_CORES)
    run = _CACHE["run"]

    w1t = np.ascontiguousarray(np.asarray(W1, np.float32).T)  # [6,64]
    w2t = np.ascontiguousarray(np.asarray(W2, np.float32).T)  # [64,64]
    w3t = np.ascontiguousarray(np.asarray(W3, np.float32).T)
    gbm = np.stack([g1, b1, g2, b2, g3, b3], axis=1).astype(np.float32)

    in_maps = []
    for c in range(N_CORES):
        in_maps.append({
            "xc": np.ascontiguousarray(x[:, c * LC:(c + 1) * LC]),
            "w1t": w1t, "w2t": w2t, "w3t": w3t, "gb": gbm,
        })

    res = run(in_maps)
    ys = [res[c]["y"] for c in range(N_CORES)]             # each [64, 1024]
    Y = np.concatenate(ys, axis=1)                         # [64, 8192]
    out = Y.reshape(C, B, M).transpose(1, 0, 2)            # [B, 64, M]
    return np.ascontiguousarray(out.astype(np.float32))



# revision 17
# speedup vs baseline: 2.6984x; 2.6984x over previous
"""GroupPointNet kernel for 8 Trainium2 NeuronCores.

Strategy (fused device pipeline):
- Host: furthest-point sampling only (jitted once on jax-CPU, numerics
  identical to the reference oracle), plus trivial input packing.
- Device (8 cores, data-parallel over the 8192 (b,m) query groups):
  KNN scores via an augmented matmul  s = 2*q.p - |p|^2  (top-20 of s
  == 20 nearest points, same value ordering as the reference's
  top_k(-d)), top-20 selection with the DVE Max8Index/MatchReplace
  instructions, point-feature gather with indirect DMA from a DRAM
  table  U[n] = (W1a+W1b)^T p_n  (so conv1 of [dp; grouped] becomes
  U[n] - W1a^T q), PE-array transposes into channel-major layout, then
  the dense pipeline: 3x (1x1 conv + LeakyReLU + train-mode BatchNorm
  with cross-core AllReduce stats) and max-pool over the 20 neighbors
  via a k-major strided access pattern.

Column layout per query tile of 128: col = k*128 + q (k-major), which
lets Max8Index output columns feed the indirect gather directly and
makes the final max-over-K a strided tensor_reduce.
"""

import numpy as np

SAMPLE_RATIO = 0.25
K = 20
SLOPE = 0.2
EPS = 1e-5

B, N, C = 4, 8192, 64
M = int(N * SAMPLE_RATIO)          # 2048
L = B * M * K                      # 163840 total columns
N_CORES = 8
GROUPS = B * M                     # 8192 (b,m) groups
GPC = GROUPS // N_CORES            # 1024 queries per core
NT = GPC // 128                    # 8 query tiles per core
LC = GPC * K                       # 20480 columns per core
TILE_COLS = 128 * K                # 2560 columns per query tile

_CACHE = {}


def _get_host_fns():
    """Jitted FPS (reference-identical numerics), built once."""
    if "hostfns" in _CACHE:
        return _CACHE["hostfns"]
    import jax
    import jax.numpy as jnp
    from jax import lax

    cpu = jax.devices("cpu")[0]

    def fps(p, m):
        B_, N_, _ = p.shape

        def step(carry, _):
            dist, last_idx = carry
            last_pt = jnp.take_along_axis(p, last_idx[:, None, None], axis=1)
            d = jnp.sum((p - last_pt) ** 2, axis=-1)
            dist = jnp.minimum(dist, d)
            nxt = jnp.argmax(dist, axis=1).astype(jnp.int32)
            return (dist, nxt), last_idx

        dist0 = jnp.full((B_, N_), 1e10, dtype=p.dtype)
        idx0 = jnp.zeros((B_,), dtype=jnp.int32)
        _, idxs = lax.scan(step, (dist0, idx0), None, length=m)
        return jnp.transpose(idxs)

    jfps = jax.jit(fps, static_argnums=1)
    _CACHE["hostfns"] = (jax, jnp, cpu, jfps)
    return _CACHE["hostfns"]


def _host_fps(p_np):
    """FPS with reference-identical numerics on jax CPU -> p1 [B,M,3]."""
    jax, jnp, cpu, jfps = _get_host_fns()
    with jax.default_device(cpu):
        p = jnp.asarray(p_np)
        idx = jfps(p, M)
        p1 = jnp.take_along_axis(p, idx[:, :, None], axis=1)
        return np.asarray(p1)


def _apply_drain_patch():
    """This walrus build rejects >1 sync wait on a CTRL-format instruction;
    split the TileContext kernel-tail drain's waits across single-wait NoOps."""
    import concourse.tile as tile_mod
    import concourse.mybir as mybir
    from concourse.vector_clock import ScopedClock

    def _split_drain_and_barrier(self, tick_clock, wait_clock):
        nc = self.nc
        drain_inst = nc.sync.drain()
        wait_clock.add_sem_waits(
            drain_inst.ins, ScopedClock({None: tick_clock.global_clock})
        )
        si = drain_inst.ins.sync_info
        if si is not None and si.on_wait and len(si.on_wait) > 1:
            waits = list(si.on_wait)
            si.on_wait = waits[:1]
            for w in waits[1:]:
                nop = nc.sync.nop(nofuse=True)
                nop.ins.sync_info = mybir.SyncInfo(on_wait=[w], on_update=[])
        nc.all_engine_barrier()
        assert self.sems is not None
        popped = nc._tile_sem_poison_stack.pop()
        assert popped is self._sem_poison
        nc.clear_and_free_semaphores(list(self.sems.allocated().values()))
        nc.all_engine_barrier()

    tile_mod.TileContext._drain_and_barrier = _split_drain_and_barrier


def _split_multi_waits(nc):
    """This walrus build allows only ONE sync wait per instruction (any
    format). Hoist extra waits onto same-engine NoOps inserted just before
    the owning instruction — in-order engines make this equivalent."""
    import concourse.mybir as mybir

    cnt = 0
    for f in nc.m.functions:
        for blk in f.blocks:
            changed = False
            out = []
            for ins in blk.instructions:
                si = ins.sync_info
                if si is not None and si.on_wait and len(si.on_wait) > 1:
                    waits = list(si.on_wait)
                    for w in waits[:-1]:
                        nop = mybir.InstNoOp(name=f"wsplit_{cnt}", ins=[], outs=[])
                        cnt += 1
                        nop.engine = ins.engine
                        nop.sync_info = mybir.SyncInfo(on_wait=[w], on_update=[])
                        out.append(nop)
                    si.on_wait = waits[-1:]
                    changed = True
                out.append(ins)
            if changed:
                blk.instructions = out
    return cnt


def _build_nc():
    import concourse.bass as bass
    import concourse.mybir as mybir
    import concourse.tile as tile
    import concourse.masks as masks

    _apply_drain_patch()
    dt = mybir.dt.float32
    u32 = mybir.dt.uint32
    Alu = mybir.AluOpType
    Act = mybir.ActivationFunctionType

    nc = bass.Bass("TRN2", target_bir_lowering=False, debug=False,
                   num_devices=N_CORES)

    ps4 = nc.dram_tensor("ps4", [4, N], dt, kind="ExternalInput")
    q4 = nc.dram_tensor("q4", [4, GPC], dt, kind="ExternalInput")
    w1sa = nc.dram_tensor("w1sa", [4, C], dt, kind="ExternalInput")
    w1aa = nc.dram_tensor("w1aa", [4, C], dt, kind="ExternalInput")
    w2t = nc.dram_tensor("w2t", [C, C], dt, kind="ExternalInput")
    w3t = nc.dram_tensor("w3t", [C, C], dt, kind="ExternalInput")
    gb = nc.dram_tensor("gb", [C, 6], dt, kind="ExternalInput")
    y = nc.dram_tensor("y", [C, GPC], dt, kind="ExternalOutput")

    inv_count = 1.0 / float(L)
    NEG = -3.0e38
    CH2 = 512                       # layer-2/3 chunk width
    NCH2 = LC // CH2                # 40 chunks

    with tile.TileContext(nc) as tc:
        with (
            tc.tile_pool(name="const", bufs=1) as cpool,
            tc.tile_pool(name="knn", bufs=1) as knn,
            tc.tile_pool(name="sel", bufs=2) as selp,
            tc.tile_pool(name="gat", bufs=2) as gat,
            tc.tile_pool(name="chunk", bufs=3) as ch,
            tc.tile_pool(name="psum", bufs=3, space="PSUM") as pp,
            tc.tile_pool(name="pst", bufs=4, space="PSUM") as pt,
            tc.tile_pool(name="stats", bufs=1) as sp,
            tc.tile_pool(name="dram", bufs=1, space="DRAM") as dram,
        ):
            # ---- constants / inputs to SBUF
            ps4s = cpool.tile([4, N], dt, tag="ps4")
            q4s = cpool.tile([4, GPC], dt, tag="q4")
            w1ss = cpool.tile([4, C], dt, tag="w1s")
            w1as = cpool.tile([4, C], dt, tag="w1a")
            w2s = cpool.tile([C, C], dt, tag="w2")
            w3s = cpool.tile([C, C], dt, tag="w3")
            gbs = cpool.tile([C, 6], dt, tag="gb")
            nc.sync.dma_start(ps4s[:], ps4[:])
            nc.sync.dma_start(q4s[:], q4[:])
            nc.sync.dma_start(w1ss[:], w1sa[:])
            nc.sync.dma_start(w1as[:], w1aa[:])
            nc.sync.dma_start(w2s[:], w2t[:])
            nc.sync.dma_start(w3s[:], w3t[:])
            nc.sync.dma_start(gbs[:], gb[:])

            ident = cpool.tile([128, 128], dt, tag="ident")
            masks.make_identity(nc, ident[:])

            # ---- U table in DRAM: U[n, :] = ps4[:, n] . w1sa  [N, C]
            # (host folds the 0.5 de-scaling of the 2p rows into w1sa, and
            # w1sa row 3 is zero so the -|p|^2 row contributes nothing)
            u_dram = dram.tile([N, C], dt, tag="udram")
            for blk in range(N // 128):
                up_t = pt.tile([128, 128], dt, tag="sm")
                up = up_t[:, :C]
                nc.tensor.matmul(up, ps4s[:, blk * 128:(blk + 1) * 128],
                                 w1ss[:], start=True, stop=True)
                us = ch.tile([128, C], dt, tag="us")
                nc.scalar.activation(us[:], up, Act.Copy, bias=0.0)
                nc.sync.dma_start(u_dram[blk * 128:(blk + 1) * 128, :], us[:])

            # ---- V [C, GPC] = w1aa^T . q_aug
            v_sb = cpool.tile([C, GPC], dt, tag="v")
            for h in range(GPC // 512):
                vp_t = pp.tile([128, 512], dt, tag="mm")
                vp = vp_t[:C, :]
                nc.tensor.matmul(vp, w1as[:],
                                 q4s[:, h * 512:(h + 1) * 512],
                                 start=True, stop=True)
                nc.scalar.activation(v_sb[:, h * 512:(h + 1) * 512], vp,
                                     Act.Copy, bias=0.0)

            # z activations live in DRAM (SBUF can't hold both the KNN
            # state and 80KB/partition slabs); streamed in chunks.
            z1 = dram.tile([C, LC], dt, tag="z1")
            z2 = dram.tile([C, LC], dt, tag="z2")
            z3 = dram.tile([C, LC], dt, tag="z3")
            ssum = sp.tile([C, NT], dt, tag="ssum1")
            qsum = sp.tile([C, NT], dt, tag="qsum1")

            # ---- per query tile: KNN scores, top-20, gather, L1
            for t in range(NT):
                d_sb = knn.tile([128, N], dt, tag="d")
                for s in range(N // 512):
                    dp_ = pp.tile([128, 512], dt, tag="mm")
                    nc.tensor.matmul(dp_[:],
                                     q4s[:, t * 128:(t + 1) * 128],
                                     ps4s[:, s * 512:(s + 1) * 512],
                                     start=True, stop=True)
                    nc.scalar.activation(d_sb[:, s * 512:(s + 1) * 512],
                                         dp_[:], Act.Copy, bias=0.0)

                mx1 = selp.tile([128, 8], dt, tag="mx1")
                mi1 = selp.tile([128, 8], u32, tag="mi1")
                mx2 = selp.tile([128, 8], dt, tag="mx2")
                mi2 = selp.tile([128, 8], u32, tag="mi2")
                mx3 = selp.tile([128, 8], dt, tag="mx3")
                mi3 = selp.tile([128, 8], u32, tag="mi3")
                nc.vector.max(out=mx1[:], in_=d_sb[:])
                nc.vector.max_index(mi1[:], mx1[:], d_sb[:])
                nc.vector.match_replace(out=d_sb[:], in_to_replace=mx1[:],
                                        in_values=d_sb[:], imm_value=NEG)
                nc.vector.max(out=mx2[:], in_=d_sb[:])
                nc.vector.max_index(mi2[:], mx2[:], d_sb[:])
                nc.vector.match_replace(out=d_sb[:], in_to_replace=mx2[:],
                                        in_values=d_sb[:], imm_value=NEG)
                nc.vector.max(out=mx3[:], in_=d_sb[:])
                nc.vector.max_index(mi3[:], mx3[:], d_sb[:])

                gU = gat.tile([128, K, C], dt, tag="gU")
                for k in range(K):
                    if k < 8:
                        idx_ap = mi1[:, k:k + 1]
                    elif k < 16:
                        idx_ap = mi2[:, k - 8:k - 7]
                    else:
                        idx_ap = mi3[:, k - 16:k - 15]
                    nc.gpsimd.indirect_dma_start(
                        out=gU[:, k, :], out_offset=None,
                        in_=u_dram[:],
                        in_offset=bass.IndirectOffsetOnAxis(ap=idx_ap, axis=0),
                    )

                # transpose each [128, C] -> [C, 128], subtract V, into a
                # tile-local slab; LeakyReLU + stats; spill to z1 DRAM
                z1t = ch.tile([C, TILE_COLS], dt, tag="z1t")
                for k in range(K):
                    tp_t = pt.tile([128, 128], dt, tag="sm")
                    tp = tp_t[:C, :]
                    nc.tensor.transpose(tp, gU[:, k, :], ident[:])
                    nc.vector.tensor_sub(z1t[:, k * 128:(k + 1) * 128], tp,
                                         v_sb[:, t * 128:(t + 1) * 128])

                nc.vector.scalar_tensor_tensor(
                    z1t[:], z1t[:], SLOPE, z1t[:],
                    Alu.mult, Alu.max, accum_out=ssum[:, t:t + 1])
                c0 = t * TILE_COLS
                nc.sync.dma_start(z1[:, c0:c0 + TILE_COLS], z1t[:])
                # square in place after the spill DMA has read z1t (WAR dep)
                nc.scalar.activation(z1t[:], z1t[:],
                                     Act.Square, accum_out=qsum[:, t:t + 1])

            def stats_and_scale(layer, s_tile, q_tile, nred, g_col, b_col):
                st = sp.tile([C, 2], dt, tag=f"st{layer}")
                nc.vector.tensor_reduce(st[:, 0:1], s_tile[:, :nred],
                                        mybir.AxisListType.X, Alu.add)
                nc.vector.tensor_reduce(st[:, 1:2], q_tile[:, :nred],
                                        mybir.AxisListType.X, Alu.add)
                cc_in = dram.tile([C, 2], dt, tag=f"ccin{layer}")
                cc_out = dram.tile([C, 2], dt, tag=f"ccout{layer}")
                nc.sync.dma_start(cc_in[:], st[:])
                nc.gpsimd.collective_compute(
                    "AllReduce", Alu.add,
                    replica_groups=[list(range(N_CORES))],
                    ins=[cc_in[:]], outs=[cc_out[:]],
                )
                gst = sp.tile([C, 2], dt, tag=f"gst{layer}")
                nc.sync.dma_start(gst[:], cc_out[:])
                mean = sp.tile([C, 1], dt, tag=f"mean{layer}")
                ex2 = sp.tile([C, 1], dt, tag=f"ex2{layer}")
                var = sp.tile([C, 1], dt, tag=f"var{layer}")
                sd = sp.tile([C, 1], dt, tag=f"sd{layer}")
                inv = sp.tile([C, 1], dt, tag=f"inv{layer}")
                scale = sp.tile([C, 1], dt, tag=f"scale{layer}")
                bias = sp.tile([C, 1], dt, tag=f"bias{layer}")
                nc.vector.tensor_scalar_mul(mean[:], gst[:, 0:1], inv_count)
                nc.vector.tensor_scalar_mul(ex2[:], gst[:, 1:2], inv_count)
                nc.vector.tensor_mul(var[:], mean[:], mean[:])
                nc.vector.tensor_sub(var[:], ex2[:], var[:])
                nc.vector.tensor_scalar_add(var[:], var[:], EPS)
                nc.scalar.activation(sd[:], var[:], Act.Sqrt, bias=0.0)
                nc.vector.reciprocal(inv[:], sd[:])
                nc.vector.tensor_mul(scale[:], g_col, inv[:])
                nc.vector.tensor_mul(bias[:], mean[:], scale[:])
                nc.vector.tensor_sub(bias[:], b_col, bias[:])
                return scale, bias

            sc1, bi1 = stats_and_scale(1, ssum, qsum, NT,
                                       gbs[:, 0:1], gbs[:, 1:2])

            def conv_layer(layer, z_in, z_out, s_tile, q_tile, w_sb, sc, bi):
                for i in range(NCH2):
                    off = i * CH2
                    xin = ch.tile([C, CH2], dt, tag="xin")
                    nc.sync.dma_start(xin[:], z_in[:, off:off + CH2])
                    xt = ch.tile([C, CH2], dt, tag="xbn")
                    nc.vector.tensor_scalar(xt[:], xin[:], sc[:], bi[:],
                                            Alu.mult, Alu.add)
                    ps_t = pp.tile([128, CH2], dt, tag="mm")
                    ps = ps_t[:C, :]
                    nc.tensor.matmul(ps, w_sb[:], xt[:],
                                     start=True, stop=True)
                    zr = ch.tile([C, CH2], dt, tag="zraw")
                    nc.scalar.activation(zr[:], ps, Act.Copy, bias=0.0)
                    nc.vector.scalar_tensor_tensor(
                        zr[:], zr[:], SLOPE, zr[:],
                        Alu.mult, Alu.max, accum_out=s_tile[:, i:i + 1])
                    nc.sync.dma_start(z_out[:, off:off + CH2], zr[:])
                    nc.scalar.activation(zr[:], zr[:], Act.Square,
                                         accum_out=q_tile[:, i:i + 1])

            ssum2 = sp.tile([C, NCH2], dt, tag="ssum2")
            qsum2 = sp.tile([C, NCH2], dt, tag="qsum2")
            conv_layer(2, z1, z2, ssum2, qsum2, w2s, sc1, bi1)
            sc2, bi2 = stats_and_scale(2, ssum2, qsum2, NCH2,
                                       gbs[:, 2:3], gbs[:, 3:4])

            ssum3 = sp.tile([C, NCH2], dt, tag="ssum3")
            qsum3 = sp.tile([C, NCH2], dt, tag="qsum3")
            conv_layer(3, z2, z3, ssum3, qsum3, w3s, sc2, bi2)
            sc3, bi3 = stats_and_scale(3, ssum3, qsum3, NCH2,
                                       gbs[:, 4:5], gbs[:, 5:6])

            # ---- BN3-apply + max-pool over K (k-major strided reduce)
            yslab = sp.tile([C, GPC], dt, tag="yslab")
            for t in range(NT):
                c0 = t * TILE_COLS
                zin = ch.tile([C, TILE_COLS], dt, tag="z3in")
                nc.sync.dma_start(zin[:], z3[:, c0:c0 + TILE_COLS])
                nc.vector.tensor_scalar(zin[:], zin[:],
                                        sc3[:], bi3[:], Alu.mult, Alu.add)
                nc.vector.tensor_reduce(
                    yslab[:, t * 128:(t + 1) * 128],
                    zin[:].rearrange("p (k q) -> p q k", k=K),
                    mybir.AxisListType.X, Alu.max)
            nc.sync.dma_start(y[:], yslab[:])

    _split_multi_waits(nc)
    return nc


def _build_runner(nc, n_cores):
    """Build the jitted PJRT executable ONCE (run_bass_via_pjrt rebuilds the
    jax.jit closure per call, forcing a retrace + relower every time)."""
    import jax
    import concourse.mybir as mybir
    from jax.sharding import Mesh, PartitionSpec
    from jax.experimental.shard_map import shard_map
    from concourse.bass2jax import (
        _bass_exec_p, install_neuronx_cc_hook, partition_id_tensor)

    install_neuronx_cc_hook()

    partition_name = (nc.partition_id_tensor.name
                      if nc.partition_id_tensor else None)
    in_names, out_names, out_avals, zero_outs = [], [], [], []
    for alloc in nc.m.functions[0].allocations:
        if not isinstance(alloc, mybir.MemoryLocationSet):
            continue
        name = alloc.memorylocations[0].name
        if alloc.kind == "ExternalInput":
            if name != partition_name:
                in_names.append(name)
        elif alloc.kind == "ExternalOutput":
            shape = tuple(alloc.tensor_shape)
            dtype = mybir.dt.np(alloc.dtype)
            out_avals.append(jax.core.ShapedArray(shape, dtype))
            out_names.append(name)
            zero_outs.append(np.zeros(shape, dtype))
    n_params = len(in_names)
    n_outs = len(out_avals)
    all_in_names = list(in_names) + list(out_names)
    if partition_name is not None:
        all_in_names.append(partition_name)
    donate = tuple(range(n_params, n_params + n_outs))

    def _body(*args):
        operands = list(args)
        if partition_name is not None:
            operands.append(partition_id_tensor())
        outs = _bass_exec_p.bind(
            *operands,
            out_avals=tuple(out_avals),
            in_names=tuple(all_in_names),
            out_names=tuple(out_names),
            lowering_input_output_aliases=(),
            sim_require_finite=True,
            sim_require_nnan=True,
            nc=nc,
        )
        return tuple(outs)

    devices = jax.devices()[:n_cores]
    mesh = Mesh(np.asarray(devices), ("core",))
    in_specs = (PartitionSpec("core"),) * (n_params + n_outs)
    out_specs = (PartitionSpec("core"),) * n_outs
    sharded = jax.jit(
        shard_map(_body, mesh=mesh, in_specs=in_specs, out_specs=out_specs,
                  check_rep=False),
        donate_argnums=donate, keep_unused=True)

    def run(in_maps):
        concat_in = [
            np.concatenate([np.asarray(in_maps[c][name])
                            for c in range(n_cores)], axis=0)
            for name in in_names
        ]
        concat_zeros = [
            np.zeros((n_cores * z.shape[0], *z.shape[1:]), z.dtype)
            for z in zero_outs
        ]
        out_arrs = sharded(*concat_in, *concat_zeros)
        return [
            {name: np.asarray(out_arrs[i]).reshape(
                n_cores, *out_avals[i].shape)[c]
             for i, name in enumerate(out_names)}
            for c in range(n_cores)
        ]

    return run


def kernel(p, W1, g1, b1, W2, g2, b2, W3, g3, b3):
    p = np.asarray(p, np.float32)
    p1 = _host_fps(p)                               # [B, M, 3]

    if "run" not in _CACHE:
        _CACHE["run"] = _build_runner(_build_nc(), N_CORES)
    run = _CACHE["run"]

    W1 = np.asarray(W1, np.float32)
    W1a = W1[:, 0:3]                                # dp part
    W1b = W1[:, 3:6]                                # grouped part
    # U is computed on-device as ps4^T . w1sa with ps4 rows (2p, -|p|^2);
    # fold the 0.5 de-scaling into the weights (exact: power-of-two scale)
    w1sa = np.zeros((4, C), np.float32)
    w1sa[0:3, :] = 0.5 * (W1a + W1b).T
    w1aa = np.zeros((4, C), np.float32)
    w1aa[0:3, :] = W1a.T
    w2t = np.ascontiguousarray(np.asarray(W2, np.float32).T)
    w3t = np.ascontiguousarray(np.asarray(W3, np.float32).T)
    gbm = np.stack([g1, b1, g2, b2, g3, b3], axis=1).astype(np.float32)

    # P_score per batch: rows (2px, 2py, 2pz, -|p|^2)
    pT = p.transpose(0, 2, 1)                       # [B, 3, N]
    psc = np.empty((B, 4, N), np.float32)
    psc[:, 0:3, :] = 2.0 * pT
    psc[:, 3, :] = -np.einsum('bdn,bdn->bn', pT, pT)

    # q_aug per core: [4, GPC] (x, y, z, 1)
    p1T = p1.transpose(0, 2, 1)                     # [B, 3, M]
    in_maps = []
    for c in range(N_CORES):
        b = c // 2
        qoff = (c % 2) * GPC
        q4 = np.empty((4, GPC), np.float32)
        q4[0:3, :] = p1T[b][:, qoff:qoff + GPC]
        q4[3, :] = 1.0
        in_maps.append({
            "ps4": np.ascontiguousarray(psc[b]),
            "q4": q4,
            "w1sa": w1sa, "w1aa": w1aa,
            "w2t": w2t, "w3t": w3t, "gb": gbm,
        })

    res = run(in_maps)
    ys = [res[c]["y"] for c in range(N_CORES)]      # each [64, 1024]
    Y = np.concatenate(ys, axis=1)                  # [64, 8192]
    out = Y.reshape(C, B, M).transpose(1, 0, 2)     # [B, 64, M]
    return np.ascontiguousarray(out.astype(np.float32))


# revision 19
# speedup vs baseline: 18.0540x; 6.6907x over previous
"""GroupPointNet kernel for 8 Trainium2 NeuronCores.

Strategy (fused device pipeline):
- Host: furthest-point sampling only (jitted once on jax-CPU, numerics
  identical to the reference oracle), plus trivial input packing.
- Device (8 cores, data-parallel over the 8192 (b,m) query groups):
  KNN scores via an augmented matmul  s = 2*q.p - |p|^2  (top-20 of s
  == 20 nearest points, same value ordering as the reference's
  top_k(-d)), top-20 selection with the DVE Max8Index/MatchReplace
  instructions, point-feature gather with indirect DMA from a DRAM
  table  U[n] = (W1a+W1b)^T p_n  (so conv1 of [dp; grouped] becomes
  U[n] - W1a^T q), PE-array transposes into channel-major layout, then
  the dense pipeline: 3x (1x1 conv + LeakyReLU + train-mode BatchNorm
  with cross-core AllReduce stats) and max-pool over the 20 neighbors
  via a k-major strided access pattern.

Column layout per query tile of 128: col = k*128 + q (k-major), which
lets Max8Index output columns feed the indirect gather directly and
makes the final max-over-K a strided tensor_reduce.
"""

import numpy as np

SAMPLE_RATIO = 0.25
K = 20
SLOPE = 0.2
EPS = 1e-5

B, N, C = 4, 8192, 64
M = int(N * SAMPLE_RATIO)          # 2048
L = B * M * K                      # 163840 total columns
N_CORES = 8
GROUPS = B * M                     # 8192 (b,m) groups
GPC = GROUPS // N_CORES            # 1024 queries per core
NT = GPC // 128                    # 8 query tiles per core
LC = GPC * K                       # 20480 columns per core
TILE_COLS = 128 * K                # 2560 columns per query tile

_CACHE = {}


def _get_host_fns():
    """Jitted FPS (reference-identical numerics), built once."""
    if "hostfns" in _CACHE:
        return _CACHE["hostfns"]
    import jax
    import jax.numpy as jnp
    from jax import lax

    cpu = jax.devices("cpu")[0]

    def fps(p, m):
        B_, N_, _ = p.shape

        def step(carry, _):
            dist, last_idx = carry
            last_pt = jnp.take_along_axis(p, last_idx[:, None, None], axis=1)
            d = jnp.sum((p - last_pt) ** 2, axis=-1)
            dist = jnp.minimum(dist, d)
            nxt = jnp.argmax(dist, axis=1).astype(jnp.int32)
            return (dist, nxt), last_idx

        dist0 = jnp.full((B_, N_), 1e10, dtype=p.dtype)
        idx0 = jnp.zeros((B_,), dtype=jnp.int32)
        _, idxs = lax.scan(step, (dist0, idx0), None, length=m)
        return jnp.transpose(idxs)

    jfps = jax.jit(fps, static_argnums=1)
    _CACHE["hostfns"] = (jax, jnp, cpu, jfps)
    return _CACHE["hostfns"]


def _host_fps_jax(p_np):
    """FPS with reference-identical numerics on jax CPU -> idx [B,M] i32."""
    jax, jnp, cpu, jfps = _get_host_fns()
    with jax.default_device(cpu):
        p = jnp.asarray(p_np)
        return np.asarray(jfps(p, M))


_FPS_C_SRC = r"""
#include <immintrin.h>
#include <string.h>

void fps(const float *px, const float *py, const float *pz,
         float *dist, int n, int m, int *out_idx) {
    for (int i = 0; i < n; i++) dist[i] = 1e10f;
    int idx = 0;
    for (int s = 0; s < m; s++) {
        out_idx[s] = idx;
        const float lx = px[idx], ly = py[idx], lz = pz[idx];
        const __m512 vlx = _mm512_set1_ps(lx);
        const __m512 vly = _mm512_set1_ps(ly);
        const __m512 vlz = _mm512_set1_ps(lz);
        __m512 vbest = _mm512_set1_ps(-1e30f);
        __m512i vbidx = _mm512_setzero_si512();
        __m512i vi = _mm512_setr_epi32(0,1,2,3,4,5,6,7,8,9,10,11,12,13,14,15);
        const __m512i vstep = _mm512_set1_epi32(16);
        for (int i = 0; i < n; i += 16) {
            __m512 x = _mm512_loadu_ps(px + i);
            __m512 y = _mm512_loadu_ps(py + i);
            __m512 z = _mm512_loadu_ps(pz + i);
            __m512 dx = _mm512_sub_ps(x, vlx);
            __m512 dy = _mm512_sub_ps(y, vly);
            __m512 dz = _mm512_sub_ps(z, vlz);
            __m512 d = _mm512_add_ps(
                _mm512_add_ps(_mm512_mul_ps(dx, dx), _mm512_mul_ps(dy, dy)),
                _mm512_mul_ps(dz, dz));
            __m512 dd = _mm512_loadu_ps(dist + i);
            __m512 nd = _mm512_min_ps(dd, d);
            _mm512_storeu_ps(dist + i, nd);
            __mmask16 gt = _mm512_cmp_ps_mask(nd, vbest, _CMP_GT_OQ);
            vbest = _mm512_mask_mov_ps(vbest, gt, nd);
            vbidx = _mm512_mask_mov_epi32(vbidx, gt, vi);
            vi = _mm512_add_epi32(vi, vstep);
        }
        float bv[16]; int bi[16];
        _mm512_storeu_ps(bv, vbest);
        _mm512_storeu_si512((__m512i *)bi, vbidx);
        float best = bv[0]; int bidx = bi[0];
        for (int l = 1; l < 16; l++) {
            if (bv[l] > best || (bv[l] == best && bi[l] < bidx)) {
                best = bv[l]; bidx = bi[l];
            }
        }
        idx = bidx;
    }
}
"""


def _get_cfps():
    """Compile (once) and load the AVX-512 FPS; None if unavailable."""
    if "cfps" in _CACHE:
        return _CACHE["cfps"]
    import ctypes, subprocess, tempfile, os
    fn = None
    try:
        d = tempfile.mkdtemp(prefix="fpsc_")
        src = os.path.join(d, "fps.c")
        so = os.path.join(d, "fps.so")
        with open(src, "w") as f:
            f.write(_FPS_C_SRC)
        subprocess.run(
            ["gcc", "-O3", "-march=native", "-ffp-contract=off",
             "-shared", "-fPIC", src, "-o", so],
            check=True, capture_output=True)
        lib = ctypes.CDLL(so)
        lib.fps.argtypes = [ctypes.POINTER(ctypes.c_float)] * 4 + \
            [ctypes.c_int, ctypes.c_int, ctypes.POINTER(ctypes.c_int)]

        def run_fps(p_np):
            idx = np.empty((B, M), np.int32)
            dist = np.empty(N, np.float32)
            fp = ctypes.POINTER(ctypes.c_float)
            ip = ctypes.POINTER(ctypes.c_int)
            for b in range(B):
                soa = np.ascontiguousarray(p_np[b].T)     # [3, N]
                lib.fps(soa[0].ctypes.data_as(fp), soa[1].ctypes.data_as(fp),
                        soa[2].ctypes.data_as(fp), dist.ctypes.data_as(fp),
                        N, M, idx[b].ctypes.data_as(ip))
            return idx
        fn = run_fps
    except Exception:
        fn = None
    _CACHE["cfps"] = fn
    return fn


def _host_fps(p_np):
    """FPS -> p1 [B,M,3]. C path validated against the jax oracle once per
    process (on the first, untimed call); fall back to jax on mismatch."""
    if "fps_use_c" not in _CACHE:
        cfps = _get_cfps()
        idx_j = _host_fps_jax(p_np)
        ok = False
        if cfps is not None:
            try:
                ok = bool(np.array_equal(cfps(p_np), idx_j))
            except Exception:
                ok = False
        _CACHE["fps_use_c"] = ok
        idx = idx_j
    elif _CACHE["fps_use_c"]:
        idx = _get_cfps()(p_np)
    else:
        idx = _host_fps_jax(p_np)
    return np.take_along_axis(p_np, idx[:, :, None], axis=1)


def _apply_drain_patch():
    """This walrus build rejects >1 sync wait on a CTRL-format instruction;
    split the TileContext kernel-tail drain's waits across single-wait NoOps."""
    import concourse.tile as tile_mod
    import concourse.mybir as mybir
    from concourse.vector_clock import ScopedClock

    def _split_drain_and_barrier(self, tick_clock, wait_clock):
        nc = self.nc
        drain_inst = nc.sync.drain()
        wait_clock.add_sem_waits(
            drain_inst.ins, ScopedClock({None: tick_clock.global_clock})
        )
        si = drain_inst.ins.sync_info
        if si is not None and si.on_wait and len(si.on_wait) > 1:
            waits = list(si.on_wait)
            si.on_wait = waits[:1]
            for w in waits[1:]:
                nop = nc.sync.nop(nofuse=True)
                nop.ins.sync_info = mybir.SyncInfo(on_wait=[w], on_update=[])
        nc.all_engine_barrier()
        assert self.sems is not None
        popped = nc._tile_sem_poison_stack.pop()
        assert popped is self._sem_poison
        nc.clear_and_free_semaphores(list(self.sems.allocated().values()))
        nc.all_engine_barrier()

    tile_mod.TileContext._drain_and_barrier = _split_drain_and_barrier


def _split_multi_waits(nc):
    """This walrus build allows only ONE sync wait per instruction (any
    format). Hoist extra waits onto same-engine NoOps inserted just before
    the owning instruction — in-order engines make this equivalent."""
    import concourse.mybir as mybir

    cnt = 0
    for f in nc.m.functions:
        for blk in f.blocks:
            changed = False
            out = []
            for ins in blk.instructions:
                si = ins.sync_info
                if si is not None and si.on_wait and len(si.on_wait) > 1:
                    waits = list(si.on_wait)
                    for w in waits[:-1]:
                        nop = mybir.InstNoOp(name=f"wsplit_{cnt}", ins=[], outs=[])
                        cnt += 1
                        nop.engine = ins.engine
                        nop.sync_info = mybir.SyncInfo(on_wait=[w], on_update=[])
                        out.append(nop)
                    si.on_wait = waits[-1:]
                    changed = True
                out.append(ins)
            if changed:
                blk.instructions = out
    return cnt


def _build_nc():
    import concourse.bass as bass
    import concourse.mybir as mybir
    import concourse.tile as tile
    import concourse.masks as masks

    _apply_drain_patch()
    dt = mybir.dt.float32
    u32 = mybir.dt.uint32
    Alu = mybir.AluOpType
    Act = mybir.ActivationFunctionType

    nc = bass.Bass("TRN2", target_bir_lowering=False, debug=False,
                   num_devices=N_CORES)

    ps4 = nc.dram_tensor("ps4", [4, N], dt, kind="ExternalInput")
    q4 = nc.dram_tensor("q4", [4, GPC], dt, kind="ExternalInput")
    w1sa = nc.dram_tensor("w1sa", [4, C], dt, kind="ExternalInput")
    w1aa = nc.dram_tensor("w1aa", [4, C], dt, kind="ExternalInput")
    w2t = nc.dram_tensor("w2t", [C, C], dt, kind="ExternalInput")
    w3t = nc.dram_tensor("w3t", [C, C], dt, kind="ExternalInput")
    gb = nc.dram_tensor("gb", [C, 6], dt, kind="ExternalInput")
    y = nc.dram_tensor("y", [C, GPC], mybir.dt.bfloat16,
                       kind="ExternalOutput")

    inv_count = 1.0 / float(L)
    NEG = -3.0e38
    CH2 = 512                       # layer-2/3 chunk width
    NCH2 = LC // CH2                # 40 chunks

    with tile.TileContext(nc) as tc:
        with (
            tc.tile_pool(name="const", bufs=1) as cpool,
            tc.tile_pool(name="knn", bufs=1) as knn,
            tc.tile_pool(name="sel", bufs=2) as selp,
            tc.tile_pool(name="gat", bufs=2) as gat,
            tc.tile_pool(name="chunk", bufs=3) as ch,
            tc.tile_pool(name="psum", bufs=3, space="PSUM") as pp,
            tc.tile_pool(name="pst", bufs=4, space="PSUM") as pt,
            tc.tile_pool(name="stats", bufs=1) as sp,
            tc.tile_pool(name="dram", bufs=1, space="DRAM") as dram,
        ):
            # ---- constants / inputs to SBUF
            ps4s = cpool.tile([4, N], dt, tag="ps4")
            q4s = cpool.tile([4, GPC], dt, tag="q4")
            w1ss = cpool.tile([4, C], dt, tag="w1s")
            w1as = cpool.tile([4, C], dt, tag="w1a")
            w2s = cpool.tile([C, C], dt, tag="w2")
            w3s = cpool.tile([C, C], dt, tag="w3")
            gbs = cpool.tile([C, 6], dt, tag="gb")
            nc.sync.dma_start(ps4s[:], ps4[:])
            nc.sync.dma_start(q4s[:], q4[:])
            nc.sync.dma_start(w1ss[:], w1sa[:])
            nc.sync.dma_start(w1as[:], w1aa[:])
            nc.sync.dma_start(w2s[:], w2t[:])
            nc.sync.dma_start(w3s[:], w3t[:])
            nc.sync.dma_start(gbs[:], gb[:])

            ident = cpool.tile([128, 128], dt, tag="ident")
            masks.make_identity(nc, ident[:])

            # ---- U table in DRAM: U[n, :] = ps4[:, n] . w1sa  [N, C]
            # (host folds the 0.5 de-scaling of the 2p rows into w1sa, and
            # w1sa row 3 is zero so the -|p|^2 row contributes nothing)
            u_dram = dram.tile([N, C], dt, tag="udram")
            for blk in range(N // 128):
                up_t = pt.tile([128, 128], dt, tag="sm")
                up = up_t[:, :C]
                nc.tensor.matmul(up, ps4s[:, blk * 128:(blk + 1) * 128],
                                 w1ss[:], start=True, stop=True)
                us = ch.tile([128, C], dt, tag="us")
                nc.scalar.activation(us[:], up, Act.Copy, bias=0.0)
                nc.sync.dma_start(u_dram[blk * 128:(blk + 1) * 128, :], us[:])

            # ---- V [C, GPC] = w1aa^T . q_aug
            v_sb = cpool.tile([C, GPC], dt, tag="v")
            for h in range(GPC // 512):
                vp_t = pp.tile([128, 512], dt, tag="mm")
                vp = vp_t[:C, :]
                nc.tensor.matmul(vp, w1as[:],
                                 q4s[:, h * 512:(h + 1) * 512],
                                 start=True, stop=True)
                nc.scalar.activation(v_sb[:, h * 512:(h + 1) * 512], vp,
                                     Act.Copy, bias=0.0)

            # z activations live in DRAM (SBUF can't hold both the KNN
            # state and 80KB/partition slabs); streamed in chunks.
            z1 = dram.tile([C, LC], dt, tag="z1")
            z2 = dram.tile([C, LC], dt, tag="z2")
            z3 = dram.tile([C, LC], dt, tag="z3")
            ssum = sp.tile([C, NT], dt, tag="ssum1")
            qsum = sp.tile([C, NT], dt, tag="qsum1")

            # ---- per query tile: KNN scores, top-20, gather, L1
            for t in range(NT):
                d_sb = knn.tile([128, N], dt, tag="d")
                for s in range(N // 512):
                    dp_ = pp.tile([128, 512], dt, tag="mm")
                    nc.tensor.matmul(dp_[:],
                                     q4s[:, t * 128:(t + 1) * 128],
                                     ps4s[:, s * 512:(s + 1) * 512],
                                     start=True, stop=True)
                    nc.scalar.activation(d_sb[:, s * 512:(s + 1) * 512],
                                         dp_[:], Act.Copy, bias=0.0)

                mx1 = selp.tile([128, 8], dt, tag="mx1")
                mi1 = selp.tile([128, 8], u32, tag="mi1")
                mx2 = selp.tile([128, 8], dt, tag="mx2")
                mi2 = selp.tile([128, 8], u32, tag="mi2")
                mx3 = selp.tile([128, 8], dt, tag="mx3")
                mi3 = selp.tile([128, 8], u32, tag="mi3")
                nc.vector.max(out=mx1[:], in_=d_sb[:])
                nc.vector.max_index(mi1[:], mx1[:], d_sb[:])
                nc.vector.match_replace(out=d_sb[:], in_to_replace=mx1[:],
                                        in_values=d_sb[:], imm_value=NEG)
                nc.vector.max(out=mx2[:], in_=d_sb[:])
                nc.vector.max_index(mi2[:], mx2[:], d_sb[:])
                nc.vector.match_replace(out=d_sb[:], in_to_replace=mx2[:],
                                        in_values=d_sb[:], imm_value=NEG)
                nc.vector.max(out=mx3[:], in_=d_sb[:])
                nc.vector.max_index(mi3[:], mx3[:], d_sb[:])

                gU = gat.tile([128, K, C], dt, tag="gU")
                for k in range(K):
                    if k < 8:
                        idx_ap = mi1[:, k:k + 1]
                    elif k < 16:
                        idx_ap = mi2[:, k - 8:k - 7]
                    else:
                        idx_ap = mi3[:, k - 16:k - 15]
                    nc.gpsimd.indirect_dma_start(
                        out=gU[:, k, :], out_offset=None,
                        in_=u_dram[:],
                        in_offset=bass.IndirectOffsetOnAxis(ap=idx_ap, axis=0),
                    )

                # transpose each [128, C] -> [C, 128], subtract V, into a
                # tile-local slab; LeakyReLU + stats; spill to z1 DRAM
                z1t = ch.tile([C, TILE_COLS], dt, tag="z1t")
                for k in range(K):
                    tp_t = pt.tile([128, 128], dt, tag="sm")
                    tp = tp_t[:C, :]
                    nc.tensor.transpose(tp, gU[:, k, :], ident[:])
                    nc.vector.tensor_sub(z1t[:, k * 128:(k + 1) * 128], tp,
                                         v_sb[:, t * 128:(t + 1) * 128])

                nc.vector.scalar_tensor_tensor(
                    z1t[:], z1t[:], SLOPE, z1t[:],
                    Alu.mult, Alu.max, accum_out=ssum[:, t:t + 1])
                c0 = t * TILE_COLS
                nc.sync.dma_start(z1[:, c0:c0 + TILE_COLS], z1t[:])
                # square in place after the spill DMA has read z1t (WAR dep)
                nc.scalar.activation(z1t[:], z1t[:],
                                     Act.Square, accum_out=qsum[:, t:t + 1])

            def stats_and_scale(layer, s_tile, q_tile, nred, g_col, b_col):
                st = sp.tile([C, 2], dt, tag=f"st{layer}")
                nc.vector.tensor_reduce(st[:, 0:1], s_tile[:, :nred],
                                        mybir.AxisListType.X, Alu.add)
                nc.vector.tensor_reduce(st[:, 1:2], q_tile[:, :nred],
                                        mybir.AxisListType.X, Alu.add)
                cc_in = dram.tile([C, 2], dt, tag=f"ccin{layer}")
                cc_out = dram.tile([C, 2], dt, tag=f"ccout{layer}")
                nc.sync.dma_start(cc_in[:], st[:])
                nc.gpsimd.collective_compute(
                    "AllReduce", Alu.add,
                    replica_groups=[list(range(N_CORES))],
                    ins=[cc_in[:]], outs=[cc_out[:]],
                )
                gst = sp.tile([C, 2], dt, tag=f"gst{layer}")
                nc.sync.dma_start(gst[:], cc_out[:])
                mean = sp.tile([C, 1], dt, tag=f"mean{layer}")
                ex2 = sp.tile([C, 1], dt, tag=f"ex2{layer}")
                var = sp.tile([C, 1], dt, tag=f"var{layer}")
                sd = sp.tile([C, 1], dt, tag=f"sd{layer}")
                inv = sp.tile([C, 1], dt, tag=f"inv{layer}")
                scale = sp.tile([C, 1], dt, tag=f"scale{layer}")
                bias = sp.tile([C, 1], dt, tag=f"bias{layer}")
                nc.vector.tensor_scalar_mul(mean[:], gst[:, 0:1], inv_count)
                nc.vector.tensor_scalar_mul(ex2[:], gst[:, 1:2], inv_count)
                nc.vector.tensor_mul(var[:], mean[:], mean[:])
                nc.vector.tensor_sub(var[:], ex2[:], var[:])
                nc.vector.tensor_scalar_add(var[:], var[:], EPS)
                nc.scalar.activation(sd[:], var[:], Act.Sqrt, bias=0.0)
                nc.vector.reciprocal(inv[:], sd[:])
                nc.vector.tensor_mul(scale[:], g_col, inv[:])
                nc.vector.tensor_mul(bias[:], mean[:], scale[:])
                nc.vector.tensor_sub(bias[:], b_col, bias[:])
                return scale, bias

            sc1, bi1 = stats_and_scale(1, ssum, qsum, NT,
                                       gbs[:, 0:1], gbs[:, 1:2])

            def conv_layer(layer, z_in, z_out, s_tile, q_tile, w_sb, sc, bi):
                for i in range(NCH2):
                    off = i * CH2
                    xin = ch.tile([C, CH2], dt, tag="xin")
                    nc.sync.dma_start(xin[:], z_in[:, off:off + CH2])
                    xt = ch.tile([C, CH2], dt, tag="xbn")
                    nc.vector.tensor_scalar(xt[:], xin[:], sc[:], bi[:],
                                            Alu.mult, Alu.add)
                    ps_t = pp.tile([128, CH2], dt, tag="mm")
                    ps = ps_t[:C, :]
                    nc.tensor.matmul(ps, w_sb[:], xt[:],
                                     start=True, stop=True)
                    zr = ch.tile([C, CH2], dt, tag="zraw")
                    nc.scalar.activation(zr[:], ps, Act.Copy, bias=0.0)
                    nc.vector.scalar_tensor_tensor(
                        zr[:], zr[:], SLOPE, zr[:],
                        Alu.mult, Alu.max, accum_out=s_tile[:, i:i + 1])
                    nc.sync.dma_start(z_out[:, off:off + CH2], zr[:])
                    nc.scalar.activation(zr[:], zr[:], Act.Square,
                                         accum_out=q_tile[:, i:i + 1])

            ssum2 = sp.tile([C, NCH2], dt, tag="ssum2")
            qsum2 = sp.tile([C, NCH2], dt, tag="qsum2")
            conv_layer(2, z1, z2, ssum2, qsum2, w2s, sc1, bi1)
            sc2, bi2 = stats_and_scale(2, ssum2, qsum2, NCH2,
                                       gbs[:, 2:3], gbs[:, 3:4])

            ssum3 = sp.tile([C, NCH2], dt, tag="ssum3")
            qsum3 = sp.tile([C, NCH2], dt, tag="qsum3")
            conv_layer(3, z2, z3, ssum3, qsum3, w3s, sc2, bi2)
            sc3, bi3 = stats_and_scale(3, ssum3, qsum3, NCH2,
                                       gbs[:, 4:5], gbs[:, 5:6])

            # ---- BN3-apply + max-pool over K (k-major strided reduce)
            yslab = sp.tile([C, GPC], mybir.dt.bfloat16, tag="yslab")
            for t in range(NT):
                c0 = t * TILE_COLS
                zin = ch.tile([C, TILE_COLS], dt, tag="z3in")
                nc.sync.dma_start(zin[:], z3[:, c0:c0 + TILE_COLS])
                nc.vector.tensor_scalar(zin[:], zin[:],
                                        sc3[:], bi3[:], Alu.mult, Alu.add)
                nc.vector.tensor_reduce(
                    yslab[:, t * 128:(t + 1) * 128],
                    zin[:].rearrange("p (k q) -> p q k", k=K),
                    mybir.AxisListType.X, Alu.max)
            nc.sync.dma_start(y[:], yslab[:])

    _split_multi_waits(nc)
    return nc


def _build_runner(nc, n_cores):
    """Build the jitted PJRT executable ONCE (run_bass_via_pjrt rebuilds the
    jax.jit closure per call, forcing a retrace + relower every time)."""
    import jax
    import concourse.mybir as mybir
    from jax.sharding import Mesh, PartitionSpec
    from jax.experimental.shard_map import shard_map
    from concourse.bass2jax import (
        _bass_exec_p, install_neuronx_cc_hook, partition_id_tensor)

    install_neuronx_cc_hook()

    partition_name = (nc.partition_id_tensor.name
                      if nc.partition_id_tensor else None)
    in_names, out_names, out_avals, zero_outs = [], [], [], []
    for alloc in nc.m.functions[0].allocations:
        if not isinstance(alloc, mybir.MemoryLocationSet):
            continue
        name = alloc.memorylocations[0].name
        if alloc.kind == "ExternalInput":
            if name != partition_name:
                in_names.append(name)
        elif alloc.kind == "ExternalOutput":
            shape = tuple(alloc.tensor_shape)
            dtype = mybir.dt.np(alloc.dtype)
            out_avals.append(jax.core.ShapedArray(shape, dtype))
            out_names.append(name)
            zero_outs.append(np.zeros(shape, dtype))
    n_params = len(in_names)
    n_outs = len(out_avals)
    all_in_names = list(in_names) + list(out_names)
    if partition_name is not None:
        all_in_names.append(partition_name)
    donate = tuple(range(n_params, n_params + n_outs))

    def _body(*args):
        operands = list(args)
        if partition_name is not None:
            operands.append(partition_id_tensor())
        outs = _bass_exec_p.bind(
            *operands,
            out_avals=tuple(out_avals),
            in_names=tuple(all_in_names),
            out_names=tuple(out_names),
            lowering_input_output_aliases=(),
            sim_require_finite=True,
            sim_require_nnan=True,
            nc=nc,
        )
        return tuple(outs)

    devices = jax.devices()[:n_cores]
    mesh = Mesh(np.asarray(devices), ("core",))
    in_specs = (PartitionSpec("core"),) * (n_params + n_outs)
    out_specs = (PartitionSpec("core"),) * n_outs
    sharded = jax.jit(
        shard_map(_body, mesh=mesh, in_specs=in_specs, out_specs=out_specs,
                  check_rep=False),
        donate_argnums=donate, keep_unused=True)

    import jax.numpy as jnp
    from jax.sharding import NamedSharding
    zshapes = [(n_cores * z.shape[0], *z.shape[1:]) for z in zero_outs]
    zdtypes = [z.dtype for z in zero_outs]
    zsharding = tuple(NamedSharding(mesh, PartitionSpec("core"))
                      for _ in zshapes)
    zfn = jax.jit(
        lambda: tuple(jnp.zeros(s_, d_) for s_, d_ in zip(zshapes, zdtypes)),
        out_shardings=zsharding)

    def run(in_maps):
        concat_in = [
            np.concatenate([np.asarray(in_maps[c][name])
                            for c in range(n_cores)], axis=0)
            for name in in_names
        ]
        concat_zeros = zfn()
        out_arrs = sharded(*concat_in, *concat_zeros)
        return [
            {name: np.asarray(out_arrs[i]).reshape(
                n_cores, *out_avals[i].shape)[c]
             for i, name in enumerate(out_names)}
            for c in range(n_cores)
        ]

    return run


def kernel(p, W1, g1, b1, W2, g2, b2, W3, g3, b3):
    p = np.asarray(p, np.float32)
    p1 = _host_fps(p)                               # [B, M, 3]

    if "run" not in _CACHE:
        _CACHE["run"] = _build_runner(_build_nc(), N_CORES)
    run = _CACHE["run"]

    W1 = np.asarray(W1, np.float32)
    W1a = W1[:, 0:3]                                # dp part
    W1b = W1[:, 3:6]                                # grouped part
    # U is computed on-device as ps4^T . w1sa with ps4 rows (2p, -|p|^2);
    # fold the 0.5 de-scaling into the weights (exact: power-of-two scale)
    w1sa = np.zeros((4, C), np.float32)
    w1sa[0:3, :] = 0.5 * (W1a + W1b).T
    w1aa = np.zeros((4, C), np.float32)
    w1aa[0:3, :] = W1a.T
    w2t = np.ascontiguousarray(np.asarray(W2, np.float32).T)
    w3t = np.ascontiguousarray(np.asarray(W3, np.float32).T)
    gbm = np.stack([g1, b1, g2, b2, g3, b3], axis=1).astype(np.float32)

    # P_score per batch: rows (2px, 2py, 2pz, -|p|^2)
    pT = p.transpose(0, 2, 1)                       # [B, 3, N]
    psc = np.empty((B, 4, N), np.float32)
    psc[:, 0:3, :] = 2.0 * pT
    psc[:, 3, :] = -np.einsum('bdn,bdn->bn', pT, pT)

    # q_aug per core: [4, GPC] (x, y, z, 1)
    p1T = p1.transpose(0, 2, 1)                     # [B, 3, M]
    in_maps = []
    for c in range(N_CORES):
        b = c // 2
        qoff = (c % 2) * GPC
        q4 = np.empty((4, GPC), np.float32)
        q4[0:3, :] = p1T[b][:, qoff:qoff + GPC]
        q4[3, :] = 1.0
        in_maps.append({
            "ps4": np.ascontiguousarray(psc[b]),
            "q4": q4,
            "w1sa": w1sa, "w1aa": w1aa,
            "w2t": w2t, "w3t": w3t, "gb": gbm,
        })

    res = run(in_maps)
    ys = [res[c]["y"].astype(np.float32)
          for c in range(N_CORES)]                  # each [64, 1024]
    Y = np.concatenate(ys, axis=1)                  # [64, 8192]
    out = Y.reshape(C, B, M).transpose(1, 0, 2)     # [B, 64, M]
    return np.ascontiguousarray(out.astype(np.float32))


# revision 24
# speedup vs baseline: 20.8333x; 1.1539x over previous
"""GroupPointNet kernel for 8 Trainium2 NeuronCores.

Strategy (fused device pipeline):
- Host: furthest-point sampling only (jitted once on jax-CPU, numerics
  identical to the reference oracle), plus trivial input packing.
- Device (8 cores, data-parallel over the 8192 (b,m) query groups):
  KNN scores via an augmented matmul  s = 2*q.p - |p|^2  (top-20 of s
  == 20 nearest points, same value ordering as the reference's
  top_k(-d)), top-20 selection with the DVE Max8Index/MatchReplace
  instructions, point-feature gather with indirect DMA from a DRAM
  table  U[n] = (W1a+W1b)^T p_n  (so conv1 of [dp; grouped] becomes
  U[n] - W1a^T q), PE-array transposes into channel-major layout, then
  the dense pipeline: 3x (1x1 conv + LeakyReLU + train-mode BatchNorm
  with cross-core AllReduce stats) and max-pool over the 20 neighbors
  via a k-major strided access pattern.

Column layout per query tile of 128: col = k*128 + q (k-major), which
lets Max8Index output columns feed the indirect gather directly and
makes the final max-over-K a strided tensor_reduce.
"""

import numpy as np

SAMPLE_RATIO = 0.25
K = 20
SLOPE = 0.2
EPS = 1e-5

B, N, C = 4, 8192, 64
M = int(N * SAMPLE_RATIO)          # 2048
L = B * M * K                      # 163840 total columns
N_CORES = 8
GROUPS = B * M                     # 8192 (b,m) groups
GPC = GROUPS // N_CORES            # 1024 queries per core
NT = GPC // 128                    # 8 query tiles per core
LC = GPC * K                       # 20480 columns per core
TILE_COLS = 128 * K                # 2560 columns per query tile

_CACHE = {}


def _get_host_fns():
    """Jitted FPS (reference-identical numerics), built once."""
    if "hostfns" in _CACHE:
        return _CACHE["hostfns"]
    import jax
    import jax.numpy as jnp
    from jax import lax

    cpu = jax.devices("cpu")[0]

    def fps(p, m):
        B_, N_, _ = p.shape

        def step(carry, _):
            dist, last_idx = carry
            last_pt = jnp.take_along_axis(p, last_idx[:, None, None], axis=1)
            d = jnp.sum((p - last_pt) ** 2, axis=-1)
            dist = jnp.minimum(dist, d)
            nxt = jnp.argmax(dist, axis=1).astype(jnp.int32)
            return (dist, nxt), last_idx

        dist0 = jnp.full((B_, N_), 1e10, dtype=p.dtype)
        idx0 = jnp.zeros((B_,), dtype=jnp.int32)
        _, idxs = lax.scan(step, (dist0, idx0), None, length=m)
        return jnp.transpose(idxs)

    jfps = jax.jit(fps, static_argnums=1)
    _CACHE["hostfns"] = (jax, jnp, cpu, jfps)
    return _CACHE["hostfns"]


def _host_fps_jax(p_np):
    """FPS with reference-identical numerics on jax CPU -> idx [B,M] i32."""
    jax, jnp, cpu, jfps = _get_host_fns()
    with jax.default_device(cpu):
        p = jnp.asarray(p_np)
        return np.asarray(jfps(p, M))


_FPS_C_SRC = r"""
#include <immintrin.h>
#include <string.h>

void fps(const float *px, const float *py, const float *pz,
         float *dist, int n, int m, int *out_idx) {
    for (int i = 0; i < n; i++) dist[i] = 1e10f;
    int idx = 0;
    for (int s = 0; s < m; s++) {
        out_idx[s] = idx;
        const float lx = px[idx], ly = py[idx], lz = pz[idx];
        const __m512 vlx = _mm512_set1_ps(lx);
        const __m512 vly = _mm512_set1_ps(ly);
        const __m512 vlz = _mm512_set1_ps(lz);
        __m512 vbest = _mm512_set1_ps(-1e30f);
        __m512i vbidx = _mm512_setzero_si512();
        __m512i vi = _mm512_setr_epi32(0,1,2,3,4,5,6,7,8,9,10,11,12,13,14,15);
        const __m512i vstep = _mm512_set1_epi32(16);
        for (int i = 0; i < n; i += 16) {
            __m512 x = _mm512_loadu_ps(px + i);
            __m512 y = _mm512_loadu_ps(py + i);
            __m512 z = _mm512_loadu_ps(pz + i);
            __m512 dx = _mm512_sub_ps(x, vlx);
            __m512 dy = _mm512_sub_ps(y, vly);
            __m512 dz = _mm512_sub_ps(z, vlz);
            __m512 d = _mm512_add_ps(
                _mm512_add_ps(_mm512_mul_ps(dx, dx), _mm512_mul_ps(dy, dy)),
                _mm512_mul_ps(dz, dz));
            __m512 dd = _mm512_loadu_ps(dist + i);
            __m512 nd = _mm512_min_ps(dd, d);
            _mm512_storeu_ps(dist + i, nd);
            __mmask16 gt = _mm512_cmp_ps_mask(nd, vbest, _CMP_GT_OQ);
            vbest = _mm512_mask_mov_ps(vbest, gt, nd);
            vbidx = _mm512_mask_mov_epi32(vbidx, gt, vi);
            vi = _mm512_add_epi32(vi, vstep);
        }
        float bv[16]; int bi[16];
        _mm512_storeu_ps(bv, vbest);
        _mm512_storeu_si512((__m512i *)bi, vbidx);
        float best = bv[0]; int bidx = bi[0];
        for (int l = 1; l < 16; l++) {
            if (bv[l] > best || (bv[l] == best && bi[l] < bidx)) {
                best = bv[l]; bidx = bi[l];
            }
        }
        idx = bidx;
    }
}
"""


def _get_cfps():
    """Compile (once) and load the AVX-512 FPS; None if unavailable."""
    if "cfps" in _CACHE:
        return _CACHE["cfps"]
    import ctypes, subprocess, tempfile, os
    fn = None
    try:
        d = tempfile.mkdtemp(prefix="fpsc_")
        src = os.path.join(d, "fps.c")
        so = os.path.join(d, "fps.so")
        with open(src, "w") as f:
            f.write(_FPS_C_SRC)
        subprocess.run(
            ["gcc", "-O3", "-march=native", "-ffp-contract=off",
             "-shared", "-fPIC", src, "-o", so],
            check=True, capture_output=True)
        lib = ctypes.CDLL(so)
        lib.fps.argtypes = [ctypes.POINTER(ctypes.c_float)] * 4 + \
            [ctypes.c_int, ctypes.c_int, ctypes.POINTER(ctypes.c_int)]

        def run_fps(p_np):
            idx = np.empty((B, M), np.int32)
            dist = np.empty(N, np.float32)
            fp = ctypes.POINTER(ctypes.c_float)
            ip = ctypes.POINTER(ctypes.c_int)
            for b in range(B):
                soa = np.ascontiguousarray(p_np[b].T)     # [3, N]
                lib.fps(soa[0].ctypes.data_as(fp), soa[1].ctypes.data_as(fp),
                        soa[2].ctypes.data_as(fp), dist.ctypes.data_as(fp),
                        N, M, idx[b].ctypes.data_as(ip))
            return idx
        fn = run_fps
    except Exception:
        fn = None
    _CACHE["cfps"] = fn
    return fn


def _host_fps(p_np):
    """FPS -> p1 [B,M,3]. C path validated against the jax oracle once per
    process (on the first, untimed call); fall back to jax on mismatch."""
    if "fps_use_c" not in _CACHE:
        cfps = _get_cfps()
        idx_j = _host_fps_jax(p_np)
        ok = False
        if cfps is not None:
            try:
                ok = bool(np.array_equal(cfps(p_np), idx_j))
            except Exception:
                ok = False
        _CACHE["fps_use_c"] = ok
        idx = idx_j
    elif _CACHE["fps_use_c"]:
        idx = _get_cfps()(p_np)
    else:
        idx = _host_fps_jax(p_np)
    return np.take_along_axis(p_np, idx[:, :, None], axis=1)


def _apply_drain_patch():
    """This walrus build rejects >1 sync wait on a CTRL-format instruction;
    split the TileContext kernel-tail drain's waits across single-wait NoOps."""
    import concourse.tile as tile_mod
    import concourse.mybir as mybir
    from concourse.vector_clock import ScopedClock

    def _split_drain_and_barrier(self, tick_clock, wait_clock):
        nc = self.nc
        drain_inst = nc.sync.drain()
        wait_clock.add_sem_waits(
            drain_inst.ins, ScopedClock({None: tick_clock.global_clock})
        )
        si = drain_inst.ins.sync_info
        if si is not None and si.on_wait and len(si.on_wait) > 1:
            waits = list(si.on_wait)
            si.on_wait = waits[:1]
            for w in waits[1:]:
                nop = nc.sync.nop(nofuse=True)
                nop.ins.sync_info = mybir.SyncInfo(on_wait=[w], on_update=[])
        nc.all_engine_barrier()
        assert self.sems is not None
        popped = nc._tile_sem_poison_stack.pop()
        assert popped is self._sem_poison
        nc.clear_and_free_semaphores(list(self.sems.allocated().values()))
        nc.all_engine_barrier()

    tile_mod.TileContext._drain_and_barrier = _split_drain_and_barrier


def _split_multi_waits(nc):
    """This walrus build allows only ONE sync wait per instruction (any
    format). Hoist extra waits onto same-engine NoOps inserted just before
    the owning instruction — in-order engines make this equivalent."""
    import concourse.mybir as mybir

    cnt = 0
    for f in nc.m.functions:
        for blk in f.blocks:
            changed = False
            out = []
            for ins in blk.instructions:
                si = ins.sync_info
                if si is not None and si.on_wait and len(si.on_wait) > 1:
                    waits = list(si.on_wait)
                    for w in waits[:-1]:
                        nop = mybir.InstNoOp(name=f"wsplit_{cnt}", ins=[], outs=[])
                        cnt += 1
                        nop.engine = ins.engine
                        nop.sync_info = mybir.SyncInfo(on_wait=[w], on_update=[])
                        out.append(nop)
                    si.on_wait = waits[-1:]
                    changed = True
                out.append(ins)
            if changed:
                blk.instructions = out
    return cnt


def _build_nc():
    import concourse.bass as bass
    import concourse.mybir as mybir
    import concourse.tile as tile
    import concourse.masks as masks

    _apply_drain_patch()
    dt = mybir.dt.float32
    u32 = mybir.dt.uint32
    Alu = mybir.AluOpType
    Act = mybir.ActivationFunctionType

    nc = bass.Bass("TRN2", target_bir_lowering=False, debug=False,
                   num_devices=N_CORES)

    # ps4 arrives as per-core halves (cores 2b/2b+1 hold batch b's halves)
    # and w2t|w3t as 16-column shards; both are AllGathered on device to
    # halve the host->device transfer.
    ps4h = nc.dram_tensor("ps4h", [4, N // 2], dt, kind="ExternalInput")
    q4 = nc.dram_tensor("q4", [4, GPC], dt, kind="ExternalInput")
    w1sa = nc.dram_tensor("w1sa", [4, C], dt, kind="ExternalInput")
    w1aa = nc.dram_tensor("w1aa", [4, C], dt, kind="ExternalInput")
    wsh = nc.dram_tensor("wsh", [C, 16], dt, kind="ExternalInput")
    gb = nc.dram_tensor("gb", [C, 6], dt, kind="ExternalInput")
    y = nc.dram_tensor("y", [C, GPC], mybir.dt.bfloat16,
                       kind="ExternalOutput")

    inv_count = 1.0 / float(L)
    NEG = -3.0e38
    CH2 = 512                       # layer-2/3 chunk width
    NCH2 = LC // CH2                # 40 chunks

    with tile.TileContext(nc) as tc:
        with (
            tc.tile_pool(name="const", bufs=1) as cpool,
            tc.tile_pool(name="knn", bufs=1) as knn,
            tc.tile_pool(name="sel", bufs=2) as selp,
            tc.tile_pool(name="gat", bufs=2) as gat,
            tc.tile_pool(name="chunk", bufs=3) as ch,
            tc.tile_pool(name="psum", bufs=3, space="PSUM") as pp,
            tc.tile_pool(name="pst", bufs=4, space="PSUM") as pt,
            tc.tile_pool(name="stats", bufs=1) as sp,
            tc.tile_pool(name="dram", bufs=1, space="DRAM") as dram,
        ):
            # ---- gather the sharded inputs across cores (collectives
            # cannot read IO tensors; stage through internal DRAM)
            ps4h_st = dram.tile([4, N // 2], dt, tag="ps4hst")
            nc.sync.dma_start(ps4h_st[:], ps4h[:])
            psf = dram.tile([2, 4, N // 2], dt, tag="psf")
            nc.gpsimd.collective_compute(
                "AllGather", Alu.bypass,
                replica_groups=[[0, 1], [2, 3], [4, 5], [6, 7]],
                ins=[ps4h_st[:]], outs=[psf[:]],
            )
            wsh_st = dram.tile([C, 16], dt, tag="wshst")
            nc.sync.dma_start(wsh_st[:], wsh[:])
            wall = dram.tile([N_CORES, C, 16], dt, tag="wall")
            nc.gpsimd.collective_compute(
                "AllGather", Alu.bypass,
                replica_groups=[list(range(N_CORES))],
                ins=[wsh_st[:]], outs=[wall[:]],
            )

            # ---- constants / inputs to SBUF
            ps4s = cpool.tile([4, N], dt, tag="ps4")
            q4s = cpool.tile([4, GPC], dt, tag="q4")
            w1ss = cpool.tile([4, C], dt, tag="w1s")
            w1as = cpool.tile([4, C], dt, tag="w1a")
            w2s = cpool.tile([C, C], dt, tag="w2")
            w3s = cpool.tile([C, C], dt, tag="w3")
            gbs = cpool.tile([C, 6], dt, tag="gb")
            # rows on partitions, halves concatenated along free dim
            nc.sync.dma_start(
                ps4s[:].rearrange("p (h e) -> p h e", h=2),
                psf[:].rearrange("h p e -> p h e"))
            nc.sync.dma_start(q4s[:], q4[:])
            nc.sync.dma_start(w1ss[:], w1sa[:])
            nc.sync.dma_start(w1as[:], w1aa[:])
            nc.sync.dma_start(
                w2s[:].rearrange("p (s e) -> p s e", s=4),
                wall[0:4].rearrange("s p e -> p s e"))
            nc.sync.dma_start(
                w3s[:].rearrange("p (s e) -> p s e", s=4),
                wall[4:8].rearrange("s p e -> p s e"))
            nc.sync.dma_start(gbs[:], gb[:])

            ident = cpool.tile([128, 128], dt, tag="ident")
            masks.make_identity(nc, ident[:])

            # ---- U table in DRAM: U[n, :] = ps4[:, n] . w1sa  [N, C]
            # (host folds the 0.5 de-scaling of the 2p rows into w1sa, and
            # w1sa row 3 is zero so the -|p|^2 row contributes nothing)
            u_dram = dram.tile([N, C], dt, tag="udram")
            for blk in range(N // 128):
                up_t = pt.tile([128, 128], dt, tag="sm")
                up = up_t[:, :C]
                nc.tensor.matmul(up, ps4s[:, blk * 128:(blk + 1) * 128],
                                 w1ss[:], start=True, stop=True)
                us = ch.tile([128, C], dt, tag="us")
                nc.scalar.activation(us[:], up, Act.Copy, bias=0.0)
                nc.sync.dma_start(u_dram[blk * 128:(blk + 1) * 128, :], us[:])

            # ---- V [C, GPC] = w1aa^T . q_aug
            v_sb = cpool.tile([C, GPC], dt, tag="v")
            for h in range(GPC // 512):
                vp_t = pp.tile([128, 512], dt, tag="mm")
                vp = vp_t[:C, :]
                nc.tensor.matmul(vp, w1as[:],
                                 q4s[:, h * 512:(h + 1) * 512],
                                 start=True, stop=True)
                nc.scalar.activation(v_sb[:, h * 512:(h + 1) * 512], vp,
                                     Act.Copy, bias=0.0)

            # z activations live in DRAM (SBUF can't hold both the KNN
            # state and 80KB/partition slabs); streamed in chunks.
            z1 = dram.tile([C, LC], dt, tag="z1")
            z2 = dram.tile([C, LC], dt, tag="z2")
            z3 = dram.tile([C, LC], dt, tag="z3")
            ssum = sp.tile([C, NT], dt, tag="ssum1")
            qsum = sp.tile([C, NT], dt, tag="qsum1")

            # ---- per query tile: KNN scores, top-20, gather, L1
            for t in range(NT):
                d_sb = knn.tile([128, N], dt, tag="d")
                for s in range(N // 512):
                    dp_ = pp.tile([128, 512], dt, tag="mm")
                    nc.tensor.matmul(dp_[:],
                                     q4s[:, t * 128:(t + 1) * 128],
                                     ps4s[:, s * 512:(s + 1) * 512],
                                     start=True, stop=True)
                    nc.scalar.activation(d_sb[:, s * 512:(s + 1) * 512],
                                         dp_[:], Act.Copy, bias=0.0)

                mx1 = selp.tile([128, 8], dt, tag="mx1")
                mi1 = selp.tile([128, 8], u32, tag="mi1")
                mx2 = selp.tile([128, 8], dt, tag="mx2")
                mi2 = selp.tile([128, 8], u32, tag="mi2")
                mx3 = selp.tile([128, 8], dt, tag="mx3")
                mi3 = selp.tile([128, 8], u32, tag="mi3")
                nc.vector.max(out=mx1[:], in_=d_sb[:])
                nc.vector.max_index(mi1[:], mx1[:], d_sb[:])
                nc.vector.match_replace(out=d_sb[:], in_to_replace=mx1[:],
                                        in_values=d_sb[:], imm_value=NEG)
                nc.vector.max(out=mx2[:], in_=d_sb[:])
                nc.vector.max_index(mi2[:], mx2[:], d_sb[:])
                nc.vector.match_replace(out=d_sb[:], in_to_replace=mx2[:],
                                        in_values=d_sb[:], imm_value=NEG)
                nc.vector.max(out=mx3[:], in_=d_sb[:])
                nc.vector.max_index(mi3[:], mx3[:], d_sb[:])

                gU = gat.tile([128, K, C], dt, tag="gU")
                for k in range(K):
                    if k < 8:
                        idx_ap = mi1[:, k:k + 1]
                    elif k < 16:
                        idx_ap = mi2[:, k - 8:k - 7]
                    else:
                        idx_ap = mi3[:, k - 16:k - 15]
                    nc.gpsimd.indirect_dma_start(
                        out=gU[:, k, :], out_offset=None,
                        in_=u_dram[:],
                        in_offset=bass.IndirectOffsetOnAxis(ap=idx_ap, axis=0),
                    )

                # transpose each [128, C] -> [C, 128], subtract V, into a
                # tile-local slab; LeakyReLU + stats; spill to z1 DRAM
                z1t = ch.tile([C, TILE_COLS], dt, tag="z1t")
                for k in range(K):
                    tp_t = pt.tile([128, 128], dt, tag="sm")
                    tp = tp_t[:C, :]
                    nc.tensor.transpose(tp, gU[:, k, :], ident[:])
                    nc.vector.tensor_sub(z1t[:, k * 128:(k + 1) * 128], tp,
                                         v_sb[:, t * 128:(t + 1) * 128])

                nc.vector.scalar_tensor_tensor(
                    z1t[:], z1t[:], SLOPE, z1t[:],
                    Alu.mult, Alu.max, accum_out=ssum[:, t:t + 1])
                c0 = t * TILE_COLS
                nc.sync.dma_start(z1[:, c0:c0 + TILE_COLS], z1t[:])
                # square in place after the spill DMA has read z1t (WAR dep)
                nc.scalar.activation(z1t[:], z1t[:],
                                     Act.Square, accum_out=qsum[:, t:t + 1])

            def stats_and_scale(layer, s_tile, q_tile, nred, g_col, b_col):
                st = sp.tile([C, 2], dt, tag=f"st{layer}")
                nc.vector.tensor_reduce(st[:, 0:1], s_tile[:, :nred],
                                        mybir.AxisListType.X, Alu.add)
                nc.vector.tensor_reduce(st[:, 1:2], q_tile[:, :nred],
                                        mybir.AxisListType.X, Alu.add)
                cc_in = dram.tile([C, 2], dt, tag=f"ccin{layer}")
                cc_out = dram.tile([C, 2], dt, tag=f"ccout{layer}")
                nc.sync.dma_start(cc_in[:], st[:])
                nc.gpsimd.collective_compute(
                    "AllReduce", Alu.add,
                    replica_groups=[list(range(N_CORES))],
                    ins=[cc_in[:]], outs=[cc_out[:]],
                )
                gst = sp.tile([C, 2], dt, tag=f"gst{layer}")
                nc.sync.dma_start(gst[:], cc_out[:])
                mean = sp.tile([C, 1], dt, tag=f"mean{layer}")
                ex2 = sp.tile([C, 1], dt, tag=f"ex2{layer}")
                var = sp.tile([C, 1], dt, tag=f"var{layer}")
                sd = sp.tile([C, 1], dt, tag=f"sd{layer}")
                inv = sp.tile([C, 1], dt, tag=f"inv{layer}")
                scale = sp.tile([C, 1], dt, tag=f"scale{layer}")
                bias = sp.tile([C, 1], dt, tag=f"bias{layer}")
                nc.vector.tensor_scalar_mul(mean[:], gst[:, 0:1], inv_count)
                nc.vector.tensor_scalar_mul(ex2[:], gst[:, 1:2], inv_count)
                nc.vector.tensor_mul(var[:], mean[:], mean[:])
                nc.vector.tensor_sub(var[:], ex2[:], var[:])
                nc.vector.tensor_scalar_add(var[:], var[:], EPS)
                nc.scalar.activation(sd[:], var[:], Act.Sqrt, bias=0.0)
                nc.vector.reciprocal(inv[:], sd[:])
                nc.vector.tensor_mul(scale[:], g_col, inv[:])
                nc.vector.tensor_mul(bias[:], mean[:], scale[:])
                nc.vector.tensor_sub(bias[:], b_col, bias[:])
                return scale, bias

            sc1, bi1 = stats_and_scale(1, ssum, qsum, NT,
                                       gbs[:, 0:1], gbs[:, 1:2])

            def conv_layer(layer, z_in, z_out, s_tile, q_tile, w_sb, sc, bi):
                for i in range(NCH2):
                    off = i * CH2
                    xin = ch.tile([C, CH2], dt, tag="xin")
                    nc.sync.dma_start(xin[:], z_in[:, off:off + CH2])
                    xt = ch.tile([C, CH2], dt, tag="xbn")
                    nc.vector.tensor_scalar(xt[:], xin[:], sc[:], bi[:],
                                            Alu.mult, Alu.add)
                    ps_t = pp.tile([128, CH2], dt, tag="mm")
                    ps = ps_t[:C, :]
                    nc.tensor.matmul(ps, w_sb[:], xt[:],
                                     start=True, stop=True)
                    zr = ch.tile([C, CH2], dt, tag="zraw")
                    nc.scalar.activation(zr[:], ps, Act.Copy, bias=0.0)
                    nc.vector.scalar_tensor_tensor(
                        zr[:], zr[:], SLOPE, zr[:],
                        Alu.mult, Alu.max, accum_out=s_tile[:, i:i + 1])
                    nc.sync.dma_start(z_out[:, off:off + CH2], zr[:])
                    nc.scalar.activation(zr[:], zr[:], Act.Square,
                                         accum_out=q_tile[:, i:i + 1])

            ssum2 = sp.tile([C, NCH2], dt, tag="ssum2")
            qsum2 = sp.tile([C, NCH2], dt, tag="qsum2")
            conv_layer(2, z1, z2, ssum2, qsum2, w2s, sc1, bi1)
            sc2, bi2 = stats_and_scale(2, ssum2, qsum2, NCH2,
                                       gbs[:, 2:3], gbs[:, 3:4])

            ssum3 = sp.tile([C, NCH2], dt, tag="ssum3")
            qsum3 = sp.tile([C, NCH2], dt, tag="qsum3")
            conv_layer(3, z2, z3, ssum3, qsum3, w3s, sc2, bi2)
            sc3, bi3 = stats_and_scale(3, ssum3, qsum3, NCH2,
                                       gbs[:, 4:5], gbs[:, 5:6])

            # ---- BN3-apply + max-pool over K (k-major strided reduce)
            yslab = sp.tile([C, GPC], mybir.dt.bfloat16, tag="yslab")
            for t in range(NT):
                c0 = t * TILE_COLS
                zin = ch.tile([C, TILE_COLS], dt, tag="z3in")
                nc.sync.dma_start(zin[:], z3[:, c0:c0 + TILE_COLS])
                nc.vector.tensor_scalar(zin[:], zin[:],
                                        sc3[:], bi3[:], Alu.mult, Alu.add)
                nc.vector.tensor_reduce(
                    yslab[:, t * 128:(t + 1) * 128],
                    zin[:].rearrange("p (k q) -> p q k", k=K),
                    mybir.AxisListType.X, Alu.max)
            nc.sync.dma_start(y[:], yslab[:])

    _split_multi_waits(nc)
    return nc


def _build_runner(nc, n_cores):
    """Build the jitted PJRT executable ONCE (run_bass_via_pjrt rebuilds the
    jax.jit closure per call, forcing a retrace + relower every time)."""
    import jax
    import concourse.mybir as mybir
    from jax.sharding import Mesh, PartitionSpec
    from jax.experimental.shard_map import shard_map
    from concourse.bass2jax import (
        _bass_exec_p, install_neuronx_cc_hook, partition_id_tensor)

    install_neuronx_cc_hook()

    partition_name = (nc.partition_id_tensor.name
                      if nc.partition_id_tensor else None)
    in_names, out_names, out_avals, zero_outs = [], [], [], []
    for alloc in nc.m.functions[0].allocations:
        if not isinstance(alloc, mybir.MemoryLocationSet):
            continue
        name = alloc.memorylocations[0].name
        if alloc.kind == "ExternalInput":
            if name != partition_name:
                in_names.append(name)
        elif alloc.kind == "ExternalOutput":
            shape = tuple(alloc.tensor_shape)
            dtype = mybir.dt.np(alloc.dtype)
            out_avals.append(jax.core.ShapedArray(shape, dtype))
            out_names.append(name)
            zero_outs.append(np.zeros(shape, dtype))
    n_params = len(in_names)
    n_outs = len(out_avals)
    all_in_names = list(in_names) + list(out_names)
    if partition_name is not None:
        all_in_names.append(partition_name)
    donate = tuple(range(n_params, n_params + n_outs))

    def _body(*args):
        operands = list(args)
        if partition_name is not None:
            operands.append(partition_id_tensor())
        outs = _bass_exec_p.bind(
            *operands,
            out_avals=tuple(out_avals),
            in_names=tuple(all_in_names),
            out_names=tuple(out_names),
            lowering_input_output_aliases=(),
            sim_require_finite=True,
            sim_require_nnan=True,
            nc=nc,
        )
        return tuple(outs)

    devices = jax.devices()[:n_cores]
    mesh = Mesh(np.asarray(devices), ("core",))
    in_specs = (PartitionSpec("core"),) * (n_params + n_outs)
    out_specs = (PartitionSpec("core"),) * n_outs
    sharded = jax.jit(
        shard_map(_body, mesh=mesh, in_specs=in_specs, out_specs=out_specs,
                  check_rep=False),
        donate_argnums=donate, keep_unused=True)

    import jax.numpy as jnp
    from jax.sharding import NamedSharding
    zshapes = [(n_cores * z.shape[0], *z.shape[1:]) for z in zero_outs]
    zdtypes = [z.dtype for z in zero_outs]
    zsharding = tuple(NamedSharding(mesh, PartitionSpec("core"))
                      for _ in zshapes)
    zfn = jax.jit(
        lambda: tuple(jnp.zeros(s_, d_) for s_, d_ in zip(zshapes, zdtypes)),
        out_shardings=zsharding)

    _CACHE["dbg"] = dict(sharded=sharded, in_names=in_names, zfn=zfn,
                         out_names=out_names, out_avals=out_avals)

    def run(in_maps):
        concat_in = [
            np.concatenate([np.asarray(in_maps[c][name])
                            for c in range(n_cores)], axis=0)
            for name in in_names
        ]
        concat_zeros = zfn()
        out_arrs = sharded(*concat_in, *concat_zeros)
        return [
            {name: np.asarray(out_arrs[i]).reshape(
                n_cores, *out_avals[i].shape)[c]
             for i, name in enumerate(out_names)}
            for c in range(n_cores)
        ]

    return run


def kernel(p, W1, g1, b1, W2, g2, b2, W3, g3, b3):
    p = np.asarray(p, np.float32)
    p1 = _host_fps(p)                               # [B, M, 3]

    if "run" not in _CACHE:
        _CACHE["run"] = _build_runner(_build_nc(), N_CORES)
    run = _CACHE["run"]

    W1 = np.asarray(W1, np.float32)
    W1a = W1[:, 0:3]                                # dp part
    W1b = W1[:, 3:6]                                # grouped part
    # U is computed on-device as ps4^T . w1sa with ps4 rows (2p, -|p|^2);
    # fold the 0.5 de-scaling into the weights (exact: power-of-two scale)
    w1sa = np.zeros((4, C), np.float32)
    w1sa[0:3, :] = 0.5 * (W1a + W1b).T
    w1aa = np.zeros((4, C), np.float32)
    w1aa[0:3, :] = W1a.T
    w2t = np.ascontiguousarray(np.asarray(W2, np.float32).T)
    w3t = np.ascontiguousarray(np.asarray(W3, np.float32).T)
    gbm = np.stack([g1, b1, g2, b2, g3, b3], axis=1).astype(np.float32)

    # P_score per batch: rows (2px, 2py, 2pz, -|p|^2)
    pT = p.transpose(0, 2, 1)                       # [B, 3, N]
    psc = np.empty((B, 4, N), np.float32)
    psc[:, 0:3, :] = 2.0 * pT
    psc[:, 3, :] = -np.einsum('bdn,bdn->bn', pT, pT)

    # q_aug per core: [4, GPC] (x, y, z, 1)
    p1T = p1.transpose(0, 2, 1)                     # [B, 3, M]
    wcat = np.concatenate([w2t, w3t], axis=1)       # [64, 128]
    in_maps = []
    for c in range(N_CORES):
        b = c // 2
        qoff = (c % 2) * GPC
        q4 = np.empty((4, GPC), np.float32)
        q4[0:3, :] = p1T[b][:, qoff:qoff + GPC]
        q4[3, :] = 1.0
        hoff = (c % 2) * (N // 2)
        in_maps.append({
            "ps4h": np.ascontiguousarray(psc[b][:, hoff:hoff + N // 2]),
            "q4": q4,
            "w1sa": w1sa, "w1aa": w1aa,
            "wsh": np.ascontiguousarray(wcat[:, 16 * c:16 * (c + 1)]),
            "gb": gbm,
        })

    res = run(in_maps)
    ys = [res[c]["y"].astype(np.float32)
          for c in range(N_CORES)]                  # each [64, 1024]
    Y = np.concatenate(ys, axis=1)                  # [64, 8192]
    out = Y.reshape(C, B, M).transpose(1, 0, 2)     # [B, 64, M]
    return np.ascontiguousarray(out.astype(np.float32))


# revision 26
# speedup vs baseline: 25.3311x; 1.2159x over previous
"""GroupPointNet kernel for 8 Trainium2 NeuronCores.

Strategy (fused device pipeline):
- Host: furthest-point sampling only (jitted once on jax-CPU, numerics
  identical to the reference oracle), plus trivial input packing.
- Device (8 cores, data-parallel over the 8192 (b,m) query groups):
  KNN scores via an augmented matmul  s = 2*q.p - |p|^2  (top-20 of s
  == 20 nearest points, same value ordering as the reference's
  top_k(-d)), top-20 selection with the DVE Max8Index/MatchReplace
  instructions, point-feature gather with indirect DMA from a DRAM
  table  U[n] = (W1a+W1b)^T p_n  (so conv1 of [dp; grouped] becomes
  U[n] - W1a^T q), PE-array transposes into channel-major layout, then
  the dense pipeline: 3x (1x1 conv + LeakyReLU + train-mode BatchNorm
  with cross-core AllReduce stats) and max-pool over the 20 neighbors
  via a k-major strided access pattern.

Column layout per query tile of 128: col = k*128 + q (k-major), which
lets Max8Index output columns feed the indirect gather directly and
makes the final max-over-K a strided tensor_reduce.
"""

import numpy as np

SAMPLE_RATIO = 0.25
K = 20
SLOPE = 0.2
EPS = 1e-5

B, N, C = 4, 8192, 64
M = int(N * SAMPLE_RATIO)          # 2048
L = B * M * K                      # 163840 total columns
N_CORES = 8
GROUPS = B * M                     # 8192 (b,m) groups
GPC = GROUPS // N_CORES            # 1024 queries per core
NT = GPC // 128                    # 8 query tiles per core
LC = GPC * K                       # 20480 columns per core
TILE_COLS = 128 * K                # 2560 columns per query tile
BLOB_SIZE = 22400                  # packed per-core input blob (f32)

_CACHE = {}


def _get_host_fns():
    """Jitted FPS (reference-identical numerics), built once."""
    if "hostfns" in _CACHE:
        return _CACHE["hostfns"]
    import jax
    import jax.numpy as jnp
    from jax import lax

    cpu = jax.devices("cpu")[0]

    def fps(p, m):
        B_, N_, _ = p.shape

        def step(carry, _):
            dist, last_idx = carry
            last_pt = jnp.take_along_axis(p, last_idx[:, None, None], axis=1)
            d = jnp.sum((p - last_pt) ** 2, axis=-1)
            dist = jnp.minimum(dist, d)
            nxt = jnp.argmax(dist, axis=1).astype(jnp.int32)
            return (dist, nxt), last_idx

        dist0 = jnp.full((B_, N_), 1e10, dtype=p.dtype)
        idx0 = jnp.zeros((B_,), dtype=jnp.int32)
        _, idxs = lax.scan(step, (dist0, idx0), None, length=m)
        return jnp.transpose(idxs)

    jfps = jax.jit(fps, static_argnums=1)
    _CACHE["hostfns"] = (jax, jnp, cpu, jfps)
    return _CACHE["hostfns"]


def _host_fps_jax(p_np):
    """FPS with reference-identical numerics on jax CPU -> idx [B,M] i32."""
    jax, jnp, cpu, jfps = _get_host_fns()
    with jax.default_device(cpu):
        p = jnp.asarray(p_np)
        return np.asarray(jfps(p, M))


_FPS_C_SRC = r"""
#include <immintrin.h>
#include <string.h>

void fps(const float *px, const float *py, const float *pz,
         float *dist, int n, int m, int *out_idx) {
    for (int i = 0; i < n; i++) dist[i] = 1e10f;
    int idx = 0;
    for (int s = 0; s < m; s++) {
        out_idx[s] = idx;
        const float lx = px[idx], ly = py[idx], lz = pz[idx];
        const __m512 vlx = _mm512_set1_ps(lx);
        const __m512 vly = _mm512_set1_ps(ly);
        const __m512 vlz = _mm512_set1_ps(lz);
        __m512 vbest = _mm512_set1_ps(-1e30f);
        __m512i vbidx = _mm512_setzero_si512();
        __m512i vi = _mm512_setr_epi32(0,1,2,3,4,5,6,7,8,9,10,11,12,13,14,15);
        const __m512i vstep = _mm512_set1_epi32(16);
        for (int i = 0; i < n; i += 16) {
            __m512 x = _mm512_loadu_ps(px + i);
            __m512 y = _mm512_loadu_ps(py + i);
            __m512 z = _mm512_loadu_ps(pz + i);
            __m512 dx = _mm512_sub_ps(x, vlx);
            __m512 dy = _mm512_sub_ps(y, vly);
            __m512 dz = _mm512_sub_ps(z, vlz);
            __m512 d = _mm512_add_ps(
                _mm512_add_ps(_mm512_mul_ps(dx, dx), _mm512_mul_ps(dy, dy)),
                _mm512_mul_ps(dz, dz));
            __m512 dd = _mm512_loadu_ps(dist + i);
            __m512 nd = _mm512_min_ps(dd, d);
            _mm512_storeu_ps(dist + i, nd);
            __mmask16 gt = _mm512_cmp_ps_mask(nd, vbest, _CMP_GT_OQ);
            vbest = _mm512_mask_mov_ps(vbest, gt, nd);
            vbidx = _mm512_mask_mov_epi32(vbidx, gt, vi);
            vi = _mm512_add_epi32(vi, vstep);
        }
        float bv[16]; int bi[16];
        _mm512_storeu_ps(bv, vbest);
        _mm512_storeu_si512((__m512i *)bi, vbidx);
        float best = bv[0]; int bidx = bi[0];
        for (int l = 1; l < 16; l++) {
            if (bv[l] > best || (bv[l] == best && bi[l] < bidx)) {
                best = bv[l]; bidx = bi[l];
            }
        }
        idx = bidx;
    }
}
"""


def _get_cfps():
    """Compile (once) and load the AVX-512 FPS; None if unavailable."""
    if "cfps" in _CACHE:
        return _CACHE["cfps"]
    import ctypes, subprocess, tempfile, os
    fn = None
    try:
        d = tempfile.mkdtemp(prefix="fpsc_")
        src = os.path.join(d, "fps.c")
        so = os.path.join(d, "fps.so")
        with open(src, "w") as f:
            f.write(_FPS_C_SRC)
        subprocess.run(
            ["gcc", "-O3", "-march=native", "-ffp-contract=off",
             "-shared", "-fPIC", src, "-o", so],
            check=True, capture_output=True)
        lib = ctypes.CDLL(so)
        lib.fps.argtypes = [ctypes.POINTER(ctypes.c_float)] * 4 + \
            [ctypes.c_int, ctypes.c_int, ctypes.POINTER(ctypes.c_int)]

        def run_fps(p_np):
            idx = np.empty((B, M), np.int32)
            dist = np.empty(N, np.float32)
            fp = ctypes.POINTER(ctypes.c_float)
            ip = ctypes.POINTER(ctypes.c_int)
            for b in range(B):
                soa = np.ascontiguousarray(p_np[b].T)     # [3, N]
                lib.fps(soa[0].ctypes.data_as(fp), soa[1].ctypes.data_as(fp),
                        soa[2].ctypes.data_as(fp), dist.ctypes.data_as(fp),
                        N, M, idx[b].ctypes.data_as(ip))
            return idx
        fn = run_fps
    except Exception:
        fn = None
    _CACHE["cfps"] = fn
    return fn


def _host_fps(p_np):
    """FPS -> p1 [B,M,3]. C path validated against the jax oracle once per
    process (on the first, untimed call); fall back to jax on mismatch."""
    if "fps_use_c" not in _CACHE:
        cfps = _get_cfps()
        idx_j = _host_fps_jax(p_np)
        ok = False
        if cfps is not None:
            try:
                ok = bool(np.array_equal(cfps(p_np), idx_j))
            except Exception:
                ok = False
        _CACHE["fps_use_c"] = ok
        idx = idx_j
    elif _CACHE["fps_use_c"]:
        idx = _get_cfps()(p_np)
    else:
        idx = _host_fps_jax(p_np)
    return np.take_along_axis(p_np, idx[:, :, None], axis=1)


def _apply_drain_patch():
    """This walrus build rejects >1 sync wait on a CTRL-format instruction;
    split the TileContext kernel-tail drain's waits across single-wait NoOps."""
    import concourse.tile as tile_mod
    import concourse.mybir as mybir
    from concourse.vector_clock import ScopedClock

    def _split_drain_and_barrier(self, tick_clock, wait_clock):
        nc = self.nc
        drain_inst = nc.sync.drain()
        wait_clock.add_sem_waits(
            drain_inst.ins, ScopedClock({None: tick_clock.global_clock})
        )
        si = drain_inst.ins.sync_info
        if si is not None and si.on_wait and len(si.on_wait) > 1:
            waits = list(si.on_wait)
            si.on_wait = waits[:1]
            for w in waits[1:]:
                nop = nc.sync.nop(nofuse=True)
                nop.ins.sync_info = mybir.SyncInfo(on_wait=[w], on_update=[])
        nc.all_engine_barrier()
        assert self.sems is not None
        popped = nc._tile_sem_poison_stack.pop()
        assert popped is self._sem_poison
        nc.clear_and_free_semaphores(list(self.sems.allocated().values()))
        nc.all_engine_barrier()

    tile_mod.TileContext._drain_and_barrier = _split_drain_and_barrier


def _split_multi_waits(nc):
    """This walrus build allows only ONE sync wait per instruction (any
    format). Hoist extra waits onto same-engine NoOps inserted just before
    the owning instruction — in-order engines make this equivalent."""
    import concourse.mybir as mybir

    cnt = 0
    for f in nc.m.functions:
        for blk in f.blocks:
            changed = False
            out = []
            for ins in blk.instructions:
                si = ins.sync_info
                if si is not None and si.on_wait and len(si.on_wait) > 1:
                    waits = list(si.on_wait)
                    for w in waits[:-1]:
                        nop = mybir.InstNoOp(name=f"wsplit_{cnt}", ins=[], outs=[])
                        cnt += 1
                        nop.engine = ins.engine
                        nop.sync_info = mybir.SyncInfo(on_wait=[w], on_update=[])
                        out.append(nop)
                    si.on_wait = waits[-1:]
                    changed = True
                out.append(ins)
            if changed:
                blk.instructions = out
    return cnt


def _build_nc():
    import concourse.bass as bass
    import concourse.mybir as mybir
    import concourse.tile as tile
    import concourse.masks as masks

    _apply_drain_patch()
    dt = mybir.dt.float32
    u32 = mybir.dt.uint32
    Alu = mybir.AluOpType
    Act = mybir.ActivationFunctionType

    nc = bass.Bass("TRN2", target_bir_lowering=False, debug=False,
                   num_devices=N_CORES)

    # All inputs packed into ONE flat blob per core (single host->device
    # transfer). Layout (f32 offsets):
    #   [0:16384]      ps4h  [4, 4096]  this core's half of its batch's
    #                  score rows (2px, 2py, 2pz, -|p|^2)
    #   [16384:20480]  q4    [4, 1024]  query rows (x, y, z, 1)
    #   [20480:20736]  w1sa  [4, 64]
    #   [20736:20992]  w1aa  [4, 64]
    #   [20992:22016]  wsh   [64, 16]   16-col shard of w2t|w3t
    #   [22016:22400]  gb    [64, 6]
    blob = nc.dram_tensor("blob", [BLOB_SIZE], dt, kind="ExternalInput")
    ps4h = blob[0:16384].rearrange("(a b) -> a b", a=4)
    q4 = blob[16384:20480].rearrange("(a b) -> a b", a=4)
    w1sa = blob[20480:20736].rearrange("(a b) -> a b", a=4)
    w1aa = blob[20736:20992].rearrange("(a b) -> a b", a=4)
    wsh = blob[20992:22016].rearrange("(a b) -> a b", a=C)
    gb = blob[22016:22400].rearrange("(a b) -> a b", a=C)
    y = nc.dram_tensor("y", [C, GPC], mybir.dt.bfloat16,
                       kind="ExternalOutput")

    inv_count = 1.0 / float(L)
    NEG = -3.0e38
    CH2 = 512                       # layer-2/3 chunk width
    NCH2 = LC // CH2                # 40 chunks

    with tile.TileContext(nc) as tc:
        with (
            tc.tile_pool(name="const", bufs=1) as cpool,
            tc.tile_pool(name="knn", bufs=1) as knn,
            tc.tile_pool(name="sel", bufs=2) as selp,
            tc.tile_pool(name="gat", bufs=2) as gat,
            tc.tile_pool(name="chunk", bufs=3) as ch,
            tc.tile_pool(name="psum", bufs=3, space="PSUM") as pp,
            tc.tile_pool(name="pst", bufs=4, space="PSUM") as pt,
            tc.tile_pool(name="stats", bufs=1) as sp,
            tc.tile_pool(name="dram", bufs=1, space="DRAM") as dram,
        ):
            # ---- gather the sharded inputs across cores (collectives
            # cannot read IO tensors; stage through internal DRAM)
            ps4h_st = dram.tile([4, N // 2], dt, tag="ps4hst")
            nc.sync.dma_start(ps4h_st[:], ps4h)
            psf = dram.tile([2, 4, N // 2], dt, tag="psf")
            nc.gpsimd.collective_compute(
                "AllGather", Alu.bypass,
                replica_groups=[[0, 1], [2, 3], [4, 5], [6, 7]],
                ins=[ps4h_st[:]], outs=[psf[:]],
            )
            wsh_st = dram.tile([C, 16], dt, tag="wshst")
            nc.sync.dma_start(wsh_st[:], wsh)
            wall = dram.tile([N_CORES, C, 16], dt, tag="wall")
            nc.gpsimd.collective_compute(
                "AllGather", Alu.bypass,
                replica_groups=[list(range(N_CORES))],
                ins=[wsh_st[:]], outs=[wall[:]],
            )

            # ---- constants / inputs to SBUF
            ps4s = cpool.tile([4, N], dt, tag="ps4")
            q4s = cpool.tile([4, GPC], dt, tag="q4")
            w1ss = cpool.tile([4, C], dt, tag="w1s")
            w1as = cpool.tile([4, C], dt, tag="w1a")
            w2s = cpool.tile([C, C], dt, tag="w2")
            w3s = cpool.tile([C, C], dt, tag="w3")
            gbs = cpool.tile([C, 6], dt, tag="gb")
            # rows on partitions, halves concatenated along free dim
            nc.sync.dma_start(
                ps4s[:].rearrange("p (h e) -> p h e", h=2),
                psf[:].rearrange("h p e -> p h e"))
            nc.sync.dma_start(q4s[:], q4)
            nc.sync.dma_start(w1ss[:], w1sa)
            nc.sync.dma_start(w1as[:], w1aa)
            nc.sync.dma_start(
                w2s[:].rearrange("p (s e) -> p s e", s=4),
                wall[0:4].rearrange("s p e -> p s e"))
            nc.sync.dma_start(
                w3s[:].rearrange("p (s e) -> p s e", s=4),
                wall[4:8].rearrange("s p e -> p s e"))
            nc.sync.dma_start(gbs[:], gb)

            ident = cpool.tile([128, 128], dt, tag="ident")
            masks.make_identity(nc, ident[:])

            # ---- U table in DRAM: U[n, :] = ps4[:, n] . w1sa  [N, C]
            # (host folds the 0.5 de-scaling of the 2p rows into w1sa, and
            # w1sa row 3 is zero so the -|p|^2 row contributes nothing)
            u_dram = dram.tile([N, C], dt, tag="udram")
            for blk in range(N // 128):
                up_t = pt.tile([128, 128], dt, tag="sm")
                up = up_t[:, :C]
                nc.tensor.matmul(up, ps4s[:, blk * 128:(blk + 1) * 128],
                                 w1ss[:], start=True, stop=True)
                us = ch.tile([128, C], dt, tag="us")
                nc.scalar.activation(us[:], up, Act.Copy, bias=0.0)
                nc.sync.dma_start(u_dram[blk * 128:(blk + 1) * 128, :], us[:])

            # ---- V [C, GPC] = w1aa^T . q_aug
            v_sb = cpool.tile([C, GPC], dt, tag="v")
            for h in range(GPC // 512):
                vp_t = pp.tile([128, 512], dt, tag="mm")
                vp = vp_t[:C, :]
                nc.tensor.matmul(vp, w1as[:],
                                 q4s[:, h * 512:(h + 1) * 512],
                                 start=True, stop=True)
                nc.scalar.activation(v_sb[:, h * 512:(h + 1) * 512], vp,
                                     Act.Copy, bias=0.0)

            # z activations live in DRAM (SBUF can't hold both the KNN
            # state and 80KB/partition slabs); streamed in chunks.
            z1 = dram.tile([C, LC], dt, tag="z1")
            z2 = dram.tile([C, LC], dt, tag="z2")
            z3 = dram.tile([C, LC], dt, tag="z3")
            ssum = sp.tile([C, NT], dt, tag="ssum1")
            qsum = sp.tile([C, NT], dt, tag="qsum1")

            # ---- per query tile: KNN scores, top-20, gather, L1
            for t in range(NT):
                d_sb = knn.tile([128, N], dt, tag="d")
                for s in range(N // 512):
                    dp_ = pp.tile([128, 512], dt, tag="mm")
                    nc.tensor.matmul(dp_[:],
                                     q4s[:, t * 128:(t + 1) * 128],
                                     ps4s[:, s * 512:(s + 1) * 512],
                                     start=True, stop=True)
                    nc.scalar.activation(d_sb[:, s * 512:(s + 1) * 512],
                                         dp_[:], Act.Copy, bias=0.0)

                mx1 = selp.tile([128, 8], dt, tag="mx1")
                mi1 = selp.tile([128, 8], u32, tag="mi1")
                mx2 = selp.tile([128, 8], dt, tag="mx2")
                mi2 = selp.tile([128, 8], u32, tag="mi2")
                mx3 = selp.tile([128, 8], dt, tag="mx3")
                mi3 = selp.tile([128, 8], u32, tag="mi3")
                nc.vector.max(out=mx1[:], in_=d_sb[:])
                nc.vector.max_index(mi1[:], mx1[:], d_sb[:])
                nc.vector.match_replace(out=d_sb[:], in_to_replace=mx1[:],
                                        in_values=d_sb[:], imm_value=NEG)
                nc.vector.max(out=mx2[:], in_=d_sb[:])
                nc.vector.max_index(mi2[:], mx2[:], d_sb[:])
                nc.vector.match_replace(out=d_sb[:], in_to_replace=mx2[:],
                                        in_values=d_sb[:], imm_value=NEG)
                nc.vector.max(out=mx3[:], in_=d_sb[:])
                nc.vector.max_index(mi3[:], mx3[:], d_sb[:])

                gU = gat.tile([128, K, C], dt, tag="gU")
                for k in range(K):
                    if k < 8:
                        idx_ap = mi1[:, k:k + 1]
                    elif k < 16:
                        idx_ap = mi2[:, k - 8:k - 7]
                    else:
                        idx_ap = mi3[:, k - 16:k - 15]
                    nc.gpsimd.indirect_dma_start(
                        out=gU[:, k, :], out_offset=None,
                        in_=u_dram[:],
                        in_offset=bass.IndirectOffsetOnAxis(ap=idx_ap, axis=0),
                    )

                # transpose each [128, C] -> [C, 128], subtract V, into a
                # tile-local slab; LeakyReLU + stats; spill to z1 DRAM
                z1t = ch.tile([C, TILE_COLS], dt, tag="z1t")
                for k in range(K):
                    tp_t = pt.tile([128, 128], dt, tag="sm")
                    tp = tp_t[:C, :]
                    nc.tensor.transpose(tp, gU[:, k, :], ident[:])
                    nc.vector.tensor_sub(z1t[:, k * 128:(k + 1) * 128], tp,
                                         v_sb[:, t * 128:(t + 1) * 128])

                nc.vector.scalar_tensor_tensor(
                    z1t[:], z1t[:], SLOPE, z1t[:],
                    Alu.mult, Alu.max, accum_out=ssum[:, t:t + 1])
                c0 = t * TILE_COLS
                nc.sync.dma_start(z1[:, c0:c0 + TILE_COLS], z1t[:])
                # square in place after the spill DMA has read z1t (WAR dep)
                nc.scalar.activation(z1t[:], z1t[:],
                                     Act.Square, accum_out=qsum[:, t:t + 1])

            def stats_and_scale(layer, s_tile, q_tile, nred, g_col, b_col):
                st = sp.tile([C, 2], dt, tag=f"st{layer}")
                nc.vector.tensor_reduce(st[:, 0:1], s_tile[:, :nred],
                                        mybir.AxisListType.X, Alu.add)
                nc.vector.tensor_reduce(st[:, 1:2], q_tile[:, :nred],
                                        mybir.AxisListType.X, Alu.add)
                cc_in = dram.tile([C, 2], dt, tag=f"ccin{layer}")
                cc_out = dram.tile([C, 2], dt, tag=f"ccout{layer}")
                nc.sync.dma_start(cc_in[:], st[:])
                nc.gpsimd.collective_compute(
                    "AllReduce", Alu.add,
                    replica_groups=[list(range(N_CORES))],
                    ins=[cc_in[:]], outs=[cc_out[:]],
                )
                gst = sp.tile([C, 2], dt, tag=f"gst{layer}")
                nc.sync.dma_start(gst[:], cc_out[:])
                mean = sp.tile([C, 1], dt, tag=f"mean{layer}")
                ex2 = sp.tile([C, 1], dt, tag=f"ex2{layer}")
                var = sp.tile([C, 1], dt, tag=f"var{layer}")
                sd = sp.tile([C, 1], dt, tag=f"sd{layer}")
                inv = sp.tile([C, 1], dt, tag=f"inv{layer}")
                scale = sp.tile([C, 1], dt, tag=f"scale{layer}")
                bias = sp.tile([C, 1], dt, tag=f"bias{layer}")
                nc.vector.tensor_scalar_mul(mean[:], gst[:, 0:1], inv_count)
                nc.vector.tensor_scalar_mul(ex2[:], gst[:, 1:2], inv_count)
                nc.vector.tensor_mul(var[:], mean[:], mean[:])
                nc.vector.tensor_sub(var[:], ex2[:], var[:])
                nc.vector.tensor_scalar_add(var[:], var[:], EPS)
                nc.scalar.activation(sd[:], var[:], Act.Sqrt, bias=0.0)
                nc.vector.reciprocal(inv[:], sd[:])
                nc.vector.tensor_mul(scale[:], g_col, inv[:])
                nc.vector.tensor_mul(bias[:], mean[:], scale[:])
                nc.vector.tensor_sub(bias[:], b_col, bias[:])
                return scale, bias

            sc1, bi1 = stats_and_scale(1, ssum, qsum, NT,
                                       gbs[:, 0:1], gbs[:, 1:2])

            def conv_layer(layer, z_in, z_out, s_tile, q_tile, w_sb, sc, bi):
                for i in range(NCH2):
                    off = i * CH2
                    xin = ch.tile([C, CH2], dt, tag="xin")
                    nc.sync.dma_start(xin[:], z_in[:, off:off + CH2])
                    xt = ch.tile([C, CH2], dt, tag="xbn")
                    nc.vector.tensor_scalar(xt[:], xin[:], sc[:], bi[:],
                                            Alu.mult, Alu.add)
                    ps_t = pp.tile([128, CH2], dt, tag="mm")
                    ps = ps_t[:C, :]
                    nc.tensor.matmul(ps, w_sb[:], xt[:],
                                     start=True, stop=True)
                    zr = ch.tile([C, CH2], dt, tag="zraw")
                    nc.scalar.activation(zr[:], ps, Act.Copy, bias=0.0)
                    nc.vector.scalar_tensor_tensor(
                        zr[:], zr[:], SLOPE, zr[:],
                        Alu.mult, Alu.max, accum_out=s_tile[:, i:i + 1])
                    nc.sync.dma_start(z_out[:, off:off + CH2], zr[:])
                    nc.scalar.activation(zr[:], zr[:], Act.Square,
                                         accum_out=q_tile[:, i:i + 1])

            ssum2 = sp.tile([C, NCH2], dt, tag="ssum2")
            qsum2 = sp.tile([C, NCH2], dt, tag="qsum2")
            conv_layer(2, z1, z2, ssum2, qsum2, w2s, sc1, bi1)
            sc2, bi2 = stats_and_scale(2, ssum2, qsum2, NCH2,
                                       gbs[:, 2:3], gbs[:, 3:4])

            ssum3 = sp.tile([C, NCH2], dt, tag="ssum3")
            qsum3 = sp.tile([C, NCH2], dt, tag="qsum3")
            conv_layer(3, z2, z3, ssum3, qsum3, w3s, sc2, bi2)
            sc3, bi3 = stats_and_scale(3, ssum3, qsum3, NCH2,
                                       gbs[:, 4:5], gbs[:, 5:6])

            # ---- BN3-apply + max-pool over K (k-major strided reduce)
            yslab = sp.tile([C, GPC], mybir.dt.bfloat16, tag="yslab")
            for t in range(NT):
                c0 = t * TILE_COLS
                zin = ch.tile([C, TILE_COLS], dt, tag="z3in")
                nc.sync.dma_start(zin[:], z3[:, c0:c0 + TILE_COLS])
                nc.vector.tensor_scalar(zin[:], zin[:],
                                        sc3[:], bi3[:], Alu.mult, Alu.add)
                nc.vector.tensor_reduce(
                    yslab[:, t * 128:(t + 1) * 128],
                    zin[:].rearrange("p (k q) -> p q k", k=K),
                    mybir.AxisListType.X, Alu.max)
            nc.sync.dma_start(y[:], yslab[:])

    _split_multi_waits(nc)
    return nc


def _build_runner(nc, n_cores):
    """Build the jitted PJRT executable ONCE (run_bass_via_pjrt rebuilds the
    jax.jit closure per call, forcing a retrace + relower every time)."""
    import jax
    import concourse.mybir as mybir
    from jax.sharding import Mesh, PartitionSpec
    from jax.experimental.shard_map import shard_map
    from concourse.bass2jax import (
        _bass_exec_p, install_neuronx_cc_hook, partition_id_tensor)

    install_neuronx_cc_hook()

    partition_name = (nc.partition_id_tensor.name
                      if nc.partition_id_tensor else None)
    in_names, out_names, out_avals, zero_outs = [], [], [], []
    for alloc in nc.m.functions[0].allocations:
        if not isinstance(alloc, mybir.MemoryLocationSet):
            continue
        name = alloc.memorylocations[0].name
        if alloc.kind == "ExternalInput":
            if name != partition_name:
                in_names.append(name)
        elif alloc.kind == "ExternalOutput":
            shape = tuple(alloc.tensor_shape)
            dtype = mybir.dt.np(alloc.dtype)
            out_avals.append(jax.core.ShapedArray(shape, dtype))
            out_names.append(name)
            zero_outs.append(np.zeros(shape, dtype))
    n_params = len(in_names)
    n_outs = len(out_avals)
    all_in_names = list(in_names) + list(out_names)
    if partition_name is not None:
        all_in_names.append(partition_name)
    donate = tuple(range(n_params, n_params + n_outs))

    def _body(*args):
        operands = list(args)
        if partition_name is not None:
            operands.append(partition_id_tensor())
        outs = _bass_exec_p.bind(
            *operands,
            out_avals=tuple(out_avals),
            in_names=tuple(all_in_names),
            out_names=tuple(out_names),
            lowering_input_output_aliases=(),
            sim_require_finite=True,
            sim_require_nnan=True,
            nc=nc,
        )
        return tuple(outs)

    devices = jax.devices()[:n_cores]
    mesh = Mesh(np.asarray(devices), ("core",))
    in_specs = (PartitionSpec("core"),) * (n_params + n_outs)
    out_specs = (PartitionSpec("core"),) * n_outs
    sharded = jax.jit(
        shard_map(_body, mesh=mesh, in_specs=in_specs, out_specs=out_specs,
                  check_rep=False),
        donate_argnums=donate, keep_unused=True)

    import jax.numpy as jnp
    from jax.sharding import NamedSharding
    zshapes = [(n_cores * z.shape[0], *z.shape[1:]) for z in zero_outs]
    zdtypes = [z.dtype for z in zero_outs]
    zsharding = tuple(NamedSharding(mesh, PartitionSpec("core"))
                      for _ in zshapes)
    zfn = jax.jit(
        lambda: tuple(jnp.zeros(s_, d_) for s_, d_ in zip(zshapes, zdtypes)),
        out_shardings=zsharding)

    _CACHE["dbg"] = dict(sharded=sharded, in_names=in_names, zfn=zfn,
                         out_names=out_names, out_avals=out_avals)

    def run(in_maps):
        concat_in = [
            np.concatenate([np.asarray(in_maps[c][name])
                            for c in range(n_cores)], axis=0)
            for name in in_names
        ]
        concat_zeros = zfn()
        out_arrs = sharded(*concat_in, *concat_zeros)
        return [
            {name: np.asarray(out_arrs[i]).reshape(
                n_cores, *out_avals[i].shape)[c]
             for i, name in enumerate(out_names)}
            for c in range(n_cores)
        ]

    return run


def kernel(p, W1, g1, b1, W2, g2, b2, W3, g3, b3):
    p = np.asarray(p, np.float32)
    p1 = _host_fps(p)                               # [B, M, 3]

    if "run" not in _CACHE:
        _CACHE["run"] = _build_runner(_build_nc(), N_CORES)
    run = _CACHE["run"]

    W1 = np.asarray(W1, np.float32)
    W1a = W1[:, 0:3]                                # dp part
    W1b = W1[:, 3:6]                                # grouped part
    # U is computed on-device as ps4^T . w1sa with ps4 rows (2p, -|p|^2);
    # fold the 0.5 de-scaling into the weights (exact: power-of-two scale)
    w1sa = np.zeros((4, C), np.float32)
    w1sa[0:3, :] = 0.5 * (W1a + W1b).T
    w1aa = np.zeros((4, C), np.float32)
    w1aa[0:3, :] = W1a.T
    w2t = np.ascontiguousarray(np.asarray(W2, np.float32).T)
    w3t = np.ascontiguousarray(np.asarray(W3, np.float32).T)
    gbm = np.stack([g1, b1, g2, b2, g3, b3], axis=1).astype(np.float32)

    # P_score per batch: rows (2px, 2py, 2pz, -|p|^2)
    pT = p.transpose(0, 2, 1)                       # [B, 3, N]
    psc = np.empty((B, 4, N), np.float32)
    psc[:, 0:3, :] = 2.0 * pT
    psc[:, 3, :] = -np.einsum('bdn,bdn->bn', pT, pT)

    # q_aug per core: [4, GPC] (x, y, z, 1)
    p1T = p1.transpose(0, 2, 1)                     # [B, 3, M]
    wcat = np.concatenate([w2t, w3t], axis=1)       # [64, 128]
    in_maps = []
    for c in range(N_CORES):
        b = c // 2
        qoff = (c % 2) * GPC
        hoff = (c % 2) * (N // 2)
        blob = np.empty(BLOB_SIZE, np.float32)
        blob[0:16384] = psc[b][:, hoff:hoff + N // 2].reshape(-1)
        q4v = blob[16384:20480].reshape(4, GPC)
        q4v[0:3, :] = p1T[b][:, qoff:qoff + GPC]
        q4v[3, :] = 1.0
        blob[20480:20736] = w1sa.reshape(-1)
        blob[20736:20992] = w1aa.reshape(-1)
        blob[20992:22016] = wcat[:, 16 * c:16 * (c + 1)].reshape(-1)
        blob[22016:22400] = gbm.reshape(-1)
        in_maps.append({"blob": blob})

    res = run(in_maps)
    ys = [res[c]["y"].astype(np.float32)
          for c in range(N_CORES)]                  # each [64, 1024]
    Y = np.concatenate(ys, axis=1)                  # [64, 8192]
    out = Y.reshape(C, B, M).transpose(1, 0, 2)     # [B, 64, M]
    return np.ascontiguousarray(out.astype(np.float32))


# revision 28
# speedup vs baseline: 25.7270x; 1.0156x over previous
"""GroupPointNet kernel for 8 Trainium2 NeuronCores.

Strategy (fused device pipeline):
- Host: furthest-point sampling only (jitted once on jax-CPU, numerics
  identical to the reference oracle), plus trivial input packing.
- Device (8 cores, data-parallel over the 8192 (b,m) query groups):
  KNN scores via an augmented matmul  s = 2*q.p - |p|^2  (top-20 of s
  == 20 nearest points, same value ordering as the reference's
  top_k(-d)), top-20 selection with the DVE Max8Index/MatchReplace
  instructions, point-feature gather with indirect DMA from a DRAM
  table  U[n] = (W1a+W1b)^T p_n  (so conv1 of [dp; grouped] becomes
  U[n] - W1a^T q), PE-array transposes into channel-major layout, then
  the dense pipeline: 3x (1x1 conv + LeakyReLU + train-mode BatchNorm
  with cross-core AllReduce stats) and max-pool over the 20 neighbors
  via a k-major strided access pattern.

Column layout per query tile of 128: col = k*128 + q (k-major), which
lets Max8Index output columns feed the indirect gather directly and
makes the final max-over-K a strided tensor_reduce.
"""

import numpy as np

SAMPLE_RATIO = 0.25
K = 20
SLOPE = 0.2
EPS = 1e-5

B, N, C = 4, 8192, 64
M = int(N * SAMPLE_RATIO)          # 2048
L = B * M * K                      # 163840 total columns
N_CORES = 8
GROUPS = B * M                     # 8192 (b,m) groups
GPC = GROUPS // N_CORES            # 1024 queries per core
NT = GPC // 128                    # 8 query tiles per core
LC = GPC * K                       # 20480 columns per core
TILE_COLS = 128 * K                # 2560 columns per query tile
BLOB_SIZE = 22400                  # packed per-core input blob (f32)

_CACHE = {}


def _get_host_fns():
    """Jitted FPS (reference-identical numerics), built once."""
    if "hostfns" in _CACHE:
        return _CACHE["hostfns"]
    import jax
    import jax.numpy as jnp
    from jax import lax

    cpu = jax.devices("cpu")[0]

    def fps(p, m):
        B_, N_, _ = p.shape

        def step(carry, _):
            dist, last_idx = carry
            last_pt = jnp.take_along_axis(p, last_idx[:, None, None], axis=1)
            d = jnp.sum((p - last_pt) ** 2, axis=-1)
            dist = jnp.minimum(dist, d)
            nxt = jnp.argmax(dist, axis=1).astype(jnp.int32)
            return (dist, nxt), last_idx

        dist0 = jnp.full((B_, N_), 1e10, dtype=p.dtype)
        idx0 = jnp.zeros((B_,), dtype=jnp.int32)
        _, idxs = lax.scan(step, (dist0, idx0), None, length=m)
        return jnp.transpose(idxs)

    jfps = jax.jit(fps, static_argnums=1)
    _CACHE["hostfns"] = (jax, jnp, cpu, jfps)
    return _CACHE["hostfns"]


def _host_fps_jax(p_np):
    """FPS with reference-identical numerics on jax CPU -> idx [B,M] i32."""
    jax, jnp, cpu, jfps = _get_host_fns()
    with jax.default_device(cpu):
        p = jnp.asarray(p_np)
        return np.asarray(jfps(p, M))


_FPS_C_SRC = r"""
#include <immintrin.h>
#include <string.h>

void fps(const float *px, const float *py, const float *pz,
         float *dist, int n, int m, int *out_idx) {
    for (int i = 0; i < n; i++) dist[i] = 1e10f;
    int idx = 0;
    for (int s = 0; s < m; s++) {
        out_idx[s] = idx;
        const float lx = px[idx], ly = py[idx], lz = pz[idx];
        const __m512 vlx = _mm512_set1_ps(lx);
        const __m512 vly = _mm512_set1_ps(ly);
        const __m512 vlz = _mm512_set1_ps(lz);
        __m512 vbest = _mm512_set1_ps(-1e30f);
        __m512i vbidx = _mm512_setzero_si512();
        __m512i vi = _mm512_setr_epi32(0,1,2,3,4,5,6,7,8,9,10,11,12,13,14,15);
        const __m512i vstep = _mm512_set1_epi32(16);
        for (int i = 0; i < n; i += 16) {
            __m512 x = _mm512_loadu_ps(px + i);
            __m512 y = _mm512_loadu_ps(py + i);
            __m512 z = _mm512_loadu_ps(pz + i);
            __m512 dx = _mm512_sub_ps(x, vlx);
            __m512 dy = _mm512_sub_ps(y, vly);
            __m512 dz = _mm512_sub_ps(z, vlz);
            __m512 d = _mm512_add_ps(
                _mm512_add_ps(_mm512_mul_ps(dx, dx), _mm512_mul_ps(dy, dy)),
                _mm512_mul_ps(dz, dz));
            __m512 dd = _mm512_loadu_ps(dist + i);
            __m512 nd = _mm512_min_ps(dd, d);
            _mm512_storeu_ps(dist + i, nd);
            __mmask16 gt = _mm512_cmp_ps_mask(nd, vbest, _CMP_GT_OQ);
            vbest = _mm512_mask_mov_ps(vbest, gt, nd);
            vbidx = _mm512_mask_mov_epi32(vbidx, gt, vi);
            vi = _mm512_add_epi32(vi, vstep);
        }
        float bv[16]; int bi[16];
        _mm512_storeu_ps(bv, vbest);
        _mm512_storeu_si512((__m512i *)bi, vbidx);
        float best = bv[0]; int bidx = bi[0];
        for (int l = 1; l < 16; l++) {
            if (bv[l] > best || (bv[l] == best && bi[l] < bidx)) {
                best = bv[l]; bidx = bi[l];
            }
        }
        idx = bidx;
    }
}
"""


def _get_cfps():
    """Compile (once) and load the AVX-512 FPS; None if unavailable."""
    if "cfps" in _CACHE:
        return _CACHE["cfps"]
    import ctypes, subprocess, tempfile, os
    fn = None
    try:
        d = tempfile.mkdtemp(prefix="fpsc_")
        src = os.path.join(d, "fps.c")
        so = os.path.join(d, "fps.so")
        with open(src, "w") as f:
            f.write(_FPS_C_SRC)
        subprocess.run(
            ["gcc", "-O3", "-march=native", "-ffp-contract=off",
             "-shared", "-fPIC", src, "-o", so],
            check=True, capture_output=True)
        lib = ctypes.CDLL(so)
        lib.fps.argtypes = [ctypes.POINTER(ctypes.c_float)] * 4 + \
            [ctypes.c_int, ctypes.c_int, ctypes.POINTER(ctypes.c_int)]

        def run_fps(p_np):
            idx = np.empty((B, M), np.int32)
            dist = np.empty(N, np.float32)
            fp = ctypes.POINTER(ctypes.c_float)
            ip = ctypes.POINTER(ctypes.c_int)
            for b in range(B):
                soa = np.ascontiguousarray(p_np[b].T)     # [3, N]
                lib.fps(soa[0].ctypes.data_as(fp), soa[1].ctypes.data_as(fp),
                        soa[2].ctypes.data_as(fp), dist.ctypes.data_as(fp),
                        N, M, idx[b].ctypes.data_as(ip))
            return idx
        fn = run_fps
    except Exception:
        fn = None
    _CACHE["cfps"] = fn
    return fn


def _host_fps(p_np):
    """FPS -> p1 [B,M,3]. C path validated against the jax oracle once per
    process (on the first, untimed call); fall back to jax on mismatch."""
    if "fps_use_c" not in _CACHE:
        cfps = _get_cfps()
        idx_j = _host_fps_jax(p_np)
        ok = False
        if cfps is not None:
            try:
                ok = bool(np.array_equal(cfps(p_np), idx_j))
            except Exception:
                ok = False
        _CACHE["fps_use_c"] = ok
        idx = idx_j
    elif _CACHE["fps_use_c"]:
        idx = _get_cfps()(p_np)
    else:
        idx = _host_fps_jax(p_np)
    return np.take_along_axis(p_np, idx[:, :, None], axis=1)


def _apply_drain_patch():
    """This walrus build rejects >1 sync wait on a CTRL-format instruction;
    split the TileContext kernel-tail drain's waits across single-wait NoOps."""
    import concourse.tile as tile_mod
    import concourse.mybir as mybir
    from concourse.vector_clock import ScopedClock

    def _split_drain_and_barrier(self, tick_clock, wait_clock):
        nc = self.nc
        drain_inst = nc.sync.drain()
        wait_clock.add_sem_waits(
            drain_inst.ins, ScopedClock({None: tick_clock.global_clock})
        )
        si = drain_inst.ins.sync_info
        if si is not None and si.on_wait and len(si.on_wait) > 1:
            waits = list(si.on_wait)
            si.on_wait = waits[:1]
            for w in waits[1:]:
                nop = nc.sync.nop(nofuse=True)
                nop.ins.sync_info = mybir.SyncInfo(on_wait=[w], on_update=[])
        nc.all_engine_barrier()
        assert self.sems is not None
        popped = nc._tile_sem_poison_stack.pop()
        assert popped is self._sem_poison
        nc.clear_and_free_semaphores(list(self.sems.allocated().values()))
        nc.all_engine_barrier()

    tile_mod.TileContext._drain_and_barrier = _split_drain_and_barrier


def _split_multi_waits(nc):
    """This walrus build allows only ONE sync wait per instruction (any
    format). Hoist extra waits onto same-engine NoOps inserted just before
    the owning instruction — in-order engines make this equivalent."""
    import concourse.mybir as mybir

    cnt = 0
    for f in nc.m.functions:
        for blk in f.blocks:
            changed = False
            out = []
            for ins in blk.instructions:
                si = ins.sync_info
                if si is not None and si.on_wait and len(si.on_wait) > 1:
                    waits = list(si.on_wait)
                    for w in waits[:-1]:
                        nop = mybir.InstNoOp(name=f"wsplit_{cnt}", ins=[], outs=[])
                        cnt += 1
                        nop.engine = ins.engine
                        nop.sync_info = mybir.SyncInfo(on_wait=[w], on_update=[])
                        out.append(nop)
                    si.on_wait = waits[-1:]
                    changed = True
                out.append(ins)
            if changed:
                blk.instructions = out
    return cnt


def _build_nc():
    import concourse.bass as bass
    import concourse.mybir as mybir
    import concourse.tile as tile
    import concourse.masks as masks

    _apply_drain_patch()
    dt = mybir.dt.float32
    u32 = mybir.dt.uint32
    Alu = mybir.AluOpType
    Act = mybir.ActivationFunctionType

    nc = bass.Bass("TRN2", target_bir_lowering=False, debug=False,
                   num_devices=N_CORES)

    # All inputs packed into ONE flat blob per core (single host->device
    # transfer). Layout (f32 offsets):
    #   [0:16384]      ps4h  [4, 4096]  this core's half of its batch's
    #                  score rows (2px, 2py, 2pz, -|p|^2)
    #   [16384:20480]  q4    [4, 1024]  query rows (x, y, z, 1)
    #   [20480:20736]  w1sa  [4, 64]
    #   [20736:20992]  w1aa  [4, 64]
    #   [20992:22016]  wsh   [64, 16]   16-col shard of w2t|w3t
    #   [22016:22400]  gb    [64, 6]
    blob = nc.dram_tensor("blob", [BLOB_SIZE], dt, kind="ExternalInput")
    ps4h = blob[0:16384].rearrange("(a b) -> a b", a=4)
    q4 = blob[16384:20480].rearrange("(a b) -> a b", a=4)
    w1sa = blob[20480:20736].rearrange("(a b) -> a b", a=4)
    w1aa = blob[20736:20992].rearrange("(a b) -> a b", a=4)
    wsh = blob[20992:22016].rearrange("(a b) -> a b", a=C)
    gb = blob[22016:22400].rearrange("(a b) -> a b", a=C)
    y = nc.dram_tensor("y", [C, GPC], mybir.dt.bfloat16,
                       kind="ExternalOutput")

    inv_count = 1.0 / float(L)
    NEG = -3.0e38
    CH2 = 512                       # layer-2/3 chunk width
    NCH2 = LC // CH2                # 40 chunks

    with tile.TileContext(nc) as tc:
        with (
            tc.tile_pool(name="const", bufs=1) as cpool,
            tc.tile_pool(name="knn", bufs=1) as knn,
            tc.tile_pool(name="sel", bufs=2) as selp,
            tc.tile_pool(name="gat", bufs=2) as gat,
            tc.tile_pool(name="chunk", bufs=3) as ch,
            tc.tile_pool(name="psum", bufs=3, space="PSUM") as pp,
            tc.tile_pool(name="pst", bufs=4, space="PSUM") as pt,
            tc.tile_pool(name="stats", bufs=1) as sp,
            tc.tile_pool(name="dram", bufs=1, space="DRAM") as dram,
        ):
            # ---- gather the sharded inputs across cores (collectives
            # cannot read IO tensors; stage through internal DRAM)
            ps4h_st = dram.tile([4, N // 2], dt, tag="ps4hst")
            nc.sync.dma_start(ps4h_st[:], ps4h)
            psf = dram.tile([2, 4, N // 2], dt, tag="psf")
            nc.gpsimd.collective_compute(
                "AllGather", Alu.bypass,
                replica_groups=[[0, 1], [2, 3], [4, 5], [6, 7]],
                ins=[ps4h_st[:]], outs=[psf[:]],
            )
            wsh_st = dram.tile([C, 16], dt, tag="wshst")
            nc.sync.dma_start(wsh_st[:], wsh)
            wall = dram.tile([N_CORES, C, 16], dt, tag="wall")
            nc.gpsimd.collective_compute(
                "AllGather", Alu.bypass,
                replica_groups=[list(range(N_CORES))],
                ins=[wsh_st[:]], outs=[wall[:]],
            )

            # ---- constants / inputs to SBUF
            ps4s = cpool.tile([4, N], dt, tag="ps4")
            q4s = cpool.tile([4, GPC], dt, tag="q4")
            w1ss = cpool.tile([4, C], dt, tag="w1s")
            w1as = cpool.tile([4, C], dt, tag="w1a")
            w2s = cpool.tile([C, C], dt, tag="w2")
            w3s = cpool.tile([C, C], dt, tag="w3")
            gbs = cpool.tile([C, 6], dt, tag="gb")
            # rows on partitions, halves concatenated along free dim
            nc.sync.dma_start(
                ps4s[:].rearrange("p (h e) -> p h e", h=2),
                psf[:].rearrange("h p e -> p h e"))
            nc.sync.dma_start(q4s[:], q4)
            nc.sync.dma_start(w1ss[:], w1sa)
            nc.sync.dma_start(w1as[:], w1aa)
            nc.sync.dma_start(
                w2s[:].rearrange("p (s e) -> p s e", s=4),
                wall[0:4].rearrange("s p e -> p s e"))
            nc.sync.dma_start(
                w3s[:].rearrange("p (s e) -> p s e", s=4),
                wall[4:8].rearrange("s p e -> p s e"))
            nc.sync.dma_start(gbs[:], gb)

            ident = cpool.tile([128, 128], dt, tag="ident")
            masks.make_identity(nc, ident[:])

            # ---- U table in DRAM: U[n, :] = ps4[:, n] . w1sa  [N, C]
            # (host folds the 0.5 de-scaling of the 2p rows into w1sa, and
            # w1sa row 3 is zero so the -|p|^2 row contributes nothing)
            u_dram = dram.tile([N, C], dt, tag="udram")
            for blk in range(N // 128):
                up_t = pt.tile([128, 128], dt, tag="sm")
                up = up_t[:, :C]
                nc.tensor.matmul(up, ps4s[:, blk * 128:(blk + 1) * 128],
                                 w1ss[:], start=True, stop=True)
                us = ch.tile([128, C], dt, tag="us")
                nc.scalar.activation(us[:], up, Act.Copy, bias=0.0)
                nc.sync.dma_start(u_dram[blk * 128:(blk + 1) * 128, :], us[:])

            # ---- V [C, GPC] = w1aa^T . q_aug
            v_sb = cpool.tile([C, GPC], dt, tag="v")
            for h in range(GPC // 512):
                vp_t = pp.tile([128, 512], dt, tag="mm")
                vp = vp_t[:C, :]
                nc.tensor.matmul(vp, w1as[:],
                                 q4s[:, h * 512:(h + 1) * 512],
                                 start=True, stop=True)
                nc.scalar.activation(v_sb[:, h * 512:(h + 1) * 512], vp,
                                     Act.Copy, bias=0.0)

            # z activations live in DRAM (SBUF can't hold both the KNN
            # state and 80KB/partition slabs); streamed in chunks.
            z1 = dram.tile([C, LC], dt, tag="z1")
            z2 = dram.tile([C, LC], dt, tag="z2")
            z3 = dram.tile([C, LC], dt, tag="z3")
            ssum = sp.tile([C, NT], dt, tag="ssum1")
            qsum = sp.tile([C, NT], dt, tag="qsum1")

            # ---- per query tile: KNN scores, top-20, gather, L1
            for t in range(NT):
                d_sb = knn.tile([128, N], dt, tag="d")
                for s in range(N // 512):
                    dp_ = pp.tile([128, 512], dt, tag="mm")
                    nc.tensor.matmul(dp_[:],
                                     q4s[:, t * 128:(t + 1) * 128],
                                     ps4s[:, s * 512:(s + 1) * 512],
                                     start=True, stop=True)
                    nc.scalar.activation(d_sb[:, s * 512:(s + 1) * 512],
                                         dp_[:], Act.Copy, bias=0.0)

                mx1 = selp.tile([128, 8], dt, tag="mx1")
                mi1 = selp.tile([128, 8], u32, tag="mi1")
                mx2 = selp.tile([128, 8], dt, tag="mx2")
                mi2 = selp.tile([128, 8], u32, tag="mi2")
                mx3 = selp.tile([128, 8], dt, tag="mx3")
                mi3 = selp.tile([128, 8], u32, tag="mi3")
                nc.vector.max(out=mx1[:], in_=d_sb[:])
                nc.vector.max_index(mi1[:], mx1[:], d_sb[:])
                nc.vector.match_replace(out=d_sb[:], in_to_replace=mx1[:],
                                        in_values=d_sb[:], imm_value=NEG)
                nc.vector.max(out=mx2[:], in_=d_sb[:])
                nc.vector.max_index(mi2[:], mx2[:], d_sb[:])
                nc.vector.match_replace(out=d_sb[:], in_to_replace=mx2[:],
                                        in_values=d_sb[:], imm_value=NEG)
                nc.vector.max(out=mx3[:], in_=d_sb[:])
                nc.vector.max_index(mi3[:], mx3[:], d_sb[:])

                gU = gat.tile([128, K, C], dt, tag="gU")
                for k in range(K):
                    if k < 8:
                        idx_ap = mi1[:, k:k + 1]
                    elif k < 16:
                        idx_ap = mi2[:, k - 8:k - 7]
                    else:
                        idx_ap = mi3[:, k - 16:k - 15]
                    nc.gpsimd.indirect_dma_start(
                        out=gU[:, k, :], out_offset=None,
                        in_=u_dram[:],
                        in_offset=bass.IndirectOffsetOnAxis(ap=idx_ap, axis=0),
                    )

                # transpose each [128, C] -> [C, 128], subtract V, into a
                # tile-local slab; LeakyReLU + stats; spill to z1 DRAM
                z1t = ch.tile([C, TILE_COLS], dt, tag="z1t")
                for k in range(K):
                    tp_t = pt.tile([128, 128], dt, tag="sm")
                    tp = tp_t[:C, :]
                    nc.tensor.transpose(tp, gU[:, k, :], ident[:])
                    nc.vector.tensor_sub(z1t[:, k * 128:(k + 1) * 128], tp,
                                         v_sb[:, t * 128:(t + 1) * 128])

                nc.vector.scalar_tensor_tensor(
                    z1t[:], z1t[:], SLOPE, z1t[:],
                    Alu.mult, Alu.max, accum_out=ssum[:, t:t + 1])
                c0 = t * TILE_COLS
                nc.sync.dma_start(z1[:, c0:c0 + TILE_COLS], z1t[:])
                # square in place after the spill DMA has read z1t (WAR dep)
                nc.scalar.activation(z1t[:], z1t[:],
                                     Act.Square, accum_out=qsum[:, t:t + 1])

            def stats_and_scale(layer, s_tile, q_tile, nred, g_col, b_col):
                st = sp.tile([C, 2], dt, tag=f"st{layer}")
                nc.vector.tensor_reduce(st[:, 0:1], s_tile[:, :nred],
                                        mybir.AxisListType.X, Alu.add)
                nc.vector.tensor_reduce(st[:, 1:2], q_tile[:, :nred],
                                        mybir.AxisListType.X, Alu.add)
                cc_in = dram.tile([C, 2], dt, tag=f"ccin{layer}")
                cc_out = dram.tile([C, 2], dt, tag=f"ccout{layer}")
                nc.sync.dma_start(cc_in[:], st[:])
                nc.gpsimd.collective_compute(
                    "AllReduce", Alu.add,
                    replica_groups=[list(range(N_CORES))],
                    ins=[cc_in[:]], outs=[cc_out[:]],
                )
                gst = sp.tile([C, 2], dt, tag=f"gst{layer}")
                nc.sync.dma_start(gst[:], cc_out[:])
                mean = sp.tile([C, 1], dt, tag=f"mean{layer}")
                ex2 = sp.tile([C, 1], dt, tag=f"ex2{layer}")
                var = sp.tile([C, 1], dt, tag=f"var{layer}")
                sd = sp.tile([C, 1], dt, tag=f"sd{layer}")
                inv = sp.tile([C, 1], dt, tag=f"inv{layer}")
                scale = sp.tile([C, 1], dt, tag=f"scale{layer}")
                bias = sp.tile([C, 1], dt, tag=f"bias{layer}")
                nc.vector.tensor_scalar_mul(mean[:], gst[:, 0:1], inv_count)
                nc.vector.tensor_scalar_mul(ex2[:], gst[:, 1:2], inv_count)
                nc.vector.tensor_mul(var[:], mean[:], mean[:])
                nc.vector.tensor_sub(var[:], ex2[:], var[:])
                nc.vector.tensor_scalar_add(var[:], var[:], EPS)
                nc.scalar.activation(sd[:], var[:], Act.Sqrt, bias=0.0)
                nc.vector.reciprocal(inv[:], sd[:])
                nc.vector.tensor_mul(scale[:], g_col, inv[:])
                nc.vector.tensor_mul(bias[:], mean[:], scale[:])
                nc.vector.tensor_sub(bias[:], b_col, bias[:])
                return scale, bias

            sc1, bi1 = stats_and_scale(1, ssum, qsum, NT,
                                       gbs[:, 0:1], gbs[:, 1:2])

            def conv_layer(layer, z_in, z_out, s_tile, q_tile, w_sb, sc, bi):
                for i in range(NCH2):
                    off = i * CH2
                    xin = ch.tile([C, CH2], dt, tag="xin")
                    nc.sync.dma_start(xin[:], z_in[:, off:off + CH2])
                    xt = ch.tile([C, CH2], dt, tag="xbn")
                    nc.vector.tensor_scalar(xt[:], xin[:], sc[:], bi[:],
                                            Alu.mult, Alu.add)
                    ps_t = pp.tile([128, CH2], dt, tag="mm")
                    ps = ps_t[:C, :]
                    nc.tensor.matmul(ps, w_sb[:], xt[:],
                                     start=True, stop=True)
                    zr = ch.tile([C, CH2], dt, tag="zraw")
                    nc.scalar.activation(zr[:], ps, Act.Copy, bias=0.0)
                    nc.vector.scalar_tensor_tensor(
                        zr[:], zr[:], SLOPE, zr[:],
                        Alu.mult, Alu.max, accum_out=s_tile[:, i:i + 1])
                    nc.sync.dma_start(z_out[:, off:off + CH2], zr[:])
                    nc.scalar.activation(zr[:], zr[:], Act.Square,
                                         accum_out=q_tile[:, i:i + 1])

            ssum2 = sp.tile([C, NCH2], dt, tag="ssum2")
            qsum2 = sp.tile([C, NCH2], dt, tag="qsum2")
            conv_layer(2, z1, z2, ssum2, qsum2, w2s, sc1, bi1)
            sc2, bi2 = stats_and_scale(2, ssum2, qsum2, NCH2,
                                       gbs[:, 2:3], gbs[:, 3:4])

            ssum3 = sp.tile([C, NCH2], dt, tag="ssum3")
            qsum3 = sp.tile([C, NCH2], dt, tag="qsum3")
            conv_layer(3, z2, z3, ssum3, qsum3, w3s, sc2, bi2)
            sc3, bi3 = stats_and_scale(3, ssum3, qsum3, NCH2,
                                       gbs[:, 4:5], gbs[:, 5:6])

            # ---- BN3-apply + max-pool over K (k-major strided reduce)
            yslab = sp.tile([C, GPC], mybir.dt.bfloat16, tag="yslab")
            for t in range(NT):
                c0 = t * TILE_COLS
                zin = ch.tile([C, TILE_COLS], dt, tag="z3in")
                nc.sync.dma_start(zin[:], z3[:, c0:c0 + TILE_COLS])
                nc.vector.tensor_scalar(zin[:], zin[:],
                                        sc3[:], bi3[:], Alu.mult, Alu.add)
                nc.vector.tensor_reduce(
                    yslab[:, t * 128:(t + 1) * 128],
                    zin[:].rearrange("p (k q) -> p q k", k=K),
                    mybir.AxisListType.X, Alu.max)
            nc.sync.dma_start(y[:], yslab[:])

    _split_multi_waits(nc)
    return nc


def _build_runner(nc, n_cores):
    """Build the jitted PJRT executable ONCE (run_bass_via_pjrt rebuilds the
    jax.jit closure per call, forcing a retrace + relower every time)."""
    import jax
    import concourse.mybir as mybir
    from jax.sharding import Mesh, PartitionSpec
    from jax.experimental.shard_map import shard_map
    from concourse.bass2jax import (
        _bass_exec_p, install_neuronx_cc_hook, partition_id_tensor)

    install_neuronx_cc_hook()

    partition_name = (nc.partition_id_tensor.name
                      if nc.partition_id_tensor else None)
    in_names, out_names, out_avals, zero_outs = [], [], [], []
    for alloc in nc.m.functions[0].allocations:
        if not isinstance(alloc, mybir.MemoryLocationSet):
            continue
        name = alloc.memorylocations[0].name
        if alloc.kind == "ExternalInput":
            if name != partition_name:
                in_names.append(name)
        elif alloc.kind == "ExternalOutput":
            shape = tuple(alloc.tensor_shape)
            dtype = mybir.dt.np(alloc.dtype)
            out_avals.append(jax.core.ShapedArray(shape, dtype))
            out_names.append(name)
            zero_outs.append(np.zeros(shape, dtype))
    n_params = len(in_names)
    n_outs = len(out_avals)
    all_in_names = list(in_names) + list(out_names)
    if partition_name is not None:
        all_in_names.append(partition_name)
    donate = tuple(range(n_params, n_params + n_outs))

    def _body(*args):
        operands = list(args)
        if partition_name is not None:
            operands.append(partition_id_tensor())
        outs = _bass_exec_p.bind(
            *operands,
            out_avals=tuple(out_avals),
            in_names=tuple(all_in_names),
            out_names=tuple(out_names),
            lowering_input_output_aliases=(),
            sim_require_finite=True,
            sim_require_nnan=True,
            nc=nc,
        )
        return tuple(outs)

    devices = jax.devices()[:n_cores]
    mesh = Mesh(np.asarray(devices), ("core",))
    in_specs = (PartitionSpec("core"),) * (n_params + n_outs)
    out_specs = (PartitionSpec("core"),) * n_outs
    sharded = jax.jit(
        shard_map(_body, mesh=mesh, in_specs=in_specs, out_specs=out_specs,
                  check_rep=False),
        donate_argnums=donate, keep_unused=True)

    import jax.numpy as jnp
    from jax.sharding import NamedSharding
    zshapes = [(n_cores * z.shape[0], *z.shape[1:]) for z in zero_outs]
    zdtypes = [z.dtype for z in zero_outs]
    zsharding = tuple(NamedSharding(mesh, PartitionSpec("core"))
                      for _ in zshapes)
    zfn = jax.jit(
        lambda: tuple(jnp.zeros(s_, d_) for s_, d_ in zip(zshapes, zdtypes)),
        out_shardings=zsharding)

    _CACHE["dbg"] = dict(sharded=sharded, in_names=in_names, zfn=zfn,
                         out_names=out_names, out_avals=out_avals)

    def run(in_maps):
        concat_in = [
            np.concatenate([np.asarray(in_maps[c][name])
                            for c in range(n_cores)], axis=0)
            for name in in_names
        ]
        concat_zeros = zfn()
        out_arrs = sharded(*concat_in, *concat_zeros)
        return [
            {name: np.asarray(out_arrs[i]).reshape(
                n_cores, *out_avals[i].shape)[c]
             for i, name in enumerate(out_names)}
            for c in range(n_cores)
        ]

    return run


def kernel(p, W1, g1, b1, W2, g2, b2, W3, g3, b3):
    p = np.asarray(p, np.float32)
    p1 = _host_fps(p)                               # [B, M, 3]

    if "run" not in _CACHE:
        _CACHE["run"] = _build_runner(_build_nc(), N_CORES)
    run = _CACHE["run"]

    W1 = np.asarray(W1, np.float32)
    W1a = W1[:, 0:3]                                # dp part
    W1b = W1[:, 3:6]                                # grouped part
    # U is computed on-device as ps4^T . w1sa with ps4 rows (2p, -|p|^2);
    # fold the 0.5 de-scaling into the weights (exact: power-of-two scale)
    w1sa = np.zeros((4, C), np.float32)
    w1sa[0:3, :] = 0.5 * (W1a + W1b).T
    w1aa = np.zeros((4, C), np.float32)
    w1aa[0:3, :] = W1a.T
    w2t = np.ascontiguousarray(np.asarray(W2, np.float32).T)
    w3t = np.ascontiguousarray(np.asarray(W3, np.float32).T)
    gbm = np.stack([g1, b1, g2, b2, g3, b3], axis=1).astype(np.float32)

    # P_score per batch: rows (2px, 2py, 2pz, -|p|^2)
    pT = p.transpose(0, 2, 1)                       # [B, 3, N]
    psc = np.empty((B, 4, N), np.float32)
    psc[:, 0:3, :] = 2.0 * pT
    psc[:, 3, :] = -np.einsum('bdn,bdn->bn', pT, pT)

    # q_aug per core: [4, GPC] (x, y, z, 1)
    p1T = p1.transpose(0, 2, 1)                     # [B, 3, M]
    wcat = np.concatenate([w2t, w3t], axis=1)       # [64, 128]
    in_maps = []
    for c in range(N_CORES):
        b = c // 2
        qoff = (c % 2) * GPC
        hoff = (c % 2) * (N // 2)
        blob = np.empty(BLOB_SIZE, np.float32)
        blob[0:16384] = psc[b][:, hoff:hoff + N // 2].reshape(-1)
        q4v = blob[16384:20480].reshape(4, GPC)
        q4v[0:3, :] = p1T[b][:, qoff:qoff + GPC]
        q4v[3, :] = 1.0
        blob[20480:20736] = w1sa.reshape(-1)
        blob[20736:20992] = w1aa.reshape(-1)
        blob[20992:22016] = wcat[:, 16 * c:16 * (c + 1)].reshape(-1)
        blob[22016:22400] = gbm.reshape(-1)
        in_maps.append({"blob": blob})

    res = run(in_maps)
    ys = [res[c]["y"].astype(np.float32)
          for c in range(N_CORES)]                  # each [64, 1024]
    Y = np.concatenate(ys, axis=1)                  # [64, 8192]
    out = Y.reshape(C, B, M).transpose(1, 0, 2)     # [B, 64, M]
    return np.ascontiguousarray(out.astype(np.float32))
